# revision 47
# baseline (speedup 1.0000x reference)
"""Llama decode block (single token) on 8 TRN2 NeuronCores, tensor-parallel.

Sharding (per sharding_hint): w_q/w_k/w_v/w_ff1 column-sharded, w_o/w_ff2
row-sharded, KV cache sharded by head (4 heads/core). AllReduce after the
attention output projection and after w_ff2. Residuals (x, x2) are added
locally after each AllReduce, so the AR payloads carry only matvec partials.

Memory-bound problem -> minimize HBM bytes and DMA descriptor count:

* Every weight is split on the host into a float16 "hi" part plus a
  float8e4 "lo" part holding 2^12*(W - hi).  3 bytes/elem instead of 4,
  with ~1e-5 effective relative error (fp16 residual quantized by fp8).
* All tensors are pre-packed on the host into the exact SBUF tile layout
  ([128 partitions, r, cols], row g(p,r)=r*128+p), so every DMA moves
  fully contiguous 16-32KB per-partition lines (the f32 baseline was
  descriptor-rate-bound at ~181 GB/s on 2KB strided descriptors).
* KV cache is fp16 (fp16 scores/attn keeps max rel err ~6e-3 vs the 2e-2
  gate; bf16 fails it).

Matvec scheme (per k-block): activation h is kept as an fp16 dual
(h1=fp16(h), h2=fp16(h-h1)) -> matmul with 2 stationary columns gives
psum rows [h1@Whi ; h2@Whi] at no extra PE cost; the lo pass uses
hs=fp16(h*2^-12) against Wlo=fp8(2^12*residual) and accumulates onto row
0, so products come out correctly scaled with no fixup pass. Row pairs
are combined either by the next matmul's contraction (q replicate, v
new-token), by PE transposes (ff1 -> silu columns), or by a SWDGE
accumulate DMA when writing the AllReduce input (wo, ff2).
"""

import math

import numpy as np
import ml_dtypes

import concourse.bass as bass
import concourse.mybir as mybir
import concourse.tile as tile
from concourse import bacc
from concourse import bass_utils

F32 = mybir.dt.float32
F16 = mybir.dt.float16
FP8 = mybir.dt.float8e4
AF = mybir.ActivationFunctionType
ALU = mybir.AluOpType

HIDDEN = 4096
N_HEADS = 32
HEAD_DIM = 128
INTERM = 11008
KV_LEN = 4096
N_CORES = 8

HEADS_PC = N_HEADS // N_CORES          # 4 heads per core
QKV_N = HEADS_PC * HEAD_DIM            # 512
FF_N = INTERM // N_CORES               # 1376
FF_NP = 1408                           # padded to 11*128
KB = HIDDEN // 128                     # 32 k-blocks of the hidden dim
SCALE = 1.0 / math.sqrt(HEAD_DIM)
LO = 2.0 ** 12                         # residual scale for the fp8 stream
ILO = 1.0 / LO

NP_FP8 = ml_dtypes.float8_e4m3

DEBUG = False          # emit intermediate tensors as extra outputs
WARM = True           # PE warm-up bursts
USE_TTR = False        # fused multiply-reduce for attention scores

# weight-stream tiling (r-blocks per DMA tile)
QKV_RT = [16, 16]                      # 32 kb in 2 tiles of [128,16,512]
WO_RT = [2, 2]                         # 4 r in 2 tiles of [128,2,4096]
FF1_RT = [6, 6, 6, 6, 6, 2]            # 32 kb over [128,r,1408] tiles
FF2_RT = [2, 2, 2, 2, 2, 1]            # 11 kb over [128,r,4096] tiles
FF1_CH = [(0, 512), (512, 512), (1024, 384)]   # ff1 psum column chunks


def _emit(nc, tc):
    i = {}

    def din(name, shape, dt=F32):
        i[name] = nc.dram_tensor(name, list(shape), dt, kind="ExternalInput").ap()

    din("x", [HIDDEN])
    din("attn_norm", [HIDDEN])
    din("ffn_norm", [HIDDEN])
    din("sinq", [2, 64])               # pre-scaled by 1/sqrt(d)
    din("cosq", [2, 64])
    din("sink", [2, 64])
    din("cosk", [2, 64])
    din("ident32", [32, 32])
    for w in ("wq", "wk", "wv"):
        din(w + "_hi", [128, KB, QKV_N], F16)
        din(w + "_lo", [128, KB, QKV_N], FP8)
    din("wo_hi", [128, HEADS_PC, HIDDEN], F16)
    din("wo_lo", [128, HEADS_PC, HIDDEN], FP8)
    din("wf1_hi", [128, KB, FF_NP], F16)
    din("wf1_lo", [128, KB, FF_NP], FP8)
    din("wf2_hi", [128, 11, HIDDEN], F16)
    din("wf2_lo", [128, 11, HIDDEN], FP8)
    din("kc", [4, 128, 8, QKV_N], F16)
    din("vc", [4, 128, 8, QKV_N], F16)
    y = nc.dram_tensor("y", [HIDDEN], F32, kind="ExternalOutput").ap()

    dbg_outs = {}

    def dbg(name, src_ap, shape):
        if not DEBUG:
            return
        d = nc.dram_tensor("dbg_" + name, list(shape), F32,
                           kind="ExternalOutput").ap()
        nc.sync.dma_start(d, src_ap)
        dbg_outs[name] = d

    with (
        tc.tile_pool(name="const", bufs=1) as cpool,
        tc.tile_pool(name="whi", bufs=5) as hpool,
        tc.tile_pool(name="wlo", bufs=4) as lpool,
        tc.tile_pool(name="kpool", bufs=2) as kpool,
        tc.tile_pool(name="vpool", bufs=2) as vpool,
        tc.tile_pool(name="sm", bufs=1) as sm,
        tc.tile_pool(name="scr", bufs=2) as scr,
        tc.tile_pool(name="psum", bufs=8, space="PSUM") as pp,
        tc.tile_pool(name="dram", bufs=1, space="DRAM") as dram,
    ):
        # ---- constants ----
        ones32 = cpool.tile([32, 1], F32)
        nc.vector.memset(ones32[:], 1.0)
        ones128 = cpool.tile([128, 1], F32)
        nc.vector.memset(ones128[:], 1.0)
        ones_r32 = cpool.tile([1, 32], F32)
        nc.vector.memset(ones_r32[:], 1.0)
        ones_r128 = cpool.tile([1, 128], F32)
        nc.vector.memset(ones_r128[:], 1.0)
        ones2_128 = cpool.tile([2, 128], F32)
        nc.vector.memset(ones2_128[:], 1.0)
        ones2_1 = cpool.tile([2, 1], F32)
        nc.vector.memset(ones2_1[:], 1.0)
        ones1_2 = cpool.tile([1, 2], F32)
        nc.vector.memset(ones1_2[:], 1.0)
        eps11 = cpool.tile([1, 1], F32)
        nc.vector.memset(eps11[:], 1e-6)
        ident32 = cpool.tile([32, 32], F32)
        nc.sync.dma_start(ident32[:], i["ident32"])
        trig = {}
        for t in ("sinq", "cosq", "sink", "cosk"):
            trig[t] = cpool.tile([2, 64], F32, name=t)
            nc.sync.dma_start(trig[t][:], i[t])

        x_rows = cpool.tile([32, 128], F32)
        nc.sync.dma_start(x_rows[:], i["x"].rearrange("(a d) -> a d", a=32))
        anorm_rows = cpool.tile([32, 128], F32)
        nc.sync.dma_start(anorm_rows[:],
                          i["attn_norm"].rearrange("(a d) -> a d", a=32))
        fnorm_rows = cpool.tile([32, 128], F32)
        nc.sync.dma_start(fnorm_rows[:],
                          i["ffn_norm"].rearrange("(a d) -> a d", a=32))

        # ---- rmsnorm -> fp16 dual columns hd[128,32,2], hs[128,32] ----
        def rmsnorm_dual(xr, nr, tag):
            sq = sm.tile([32, 128], F32, name=f"sq_{tag}")
            ssq = sm.tile([32, 1], F32, name=f"ssq_{tag}")
            nc.scalar.activation(sq[:], xr[:], AF.Square, accum_out=ssq[:])
            ms_ps = pp.tile([1, 1], F32, name=f"ms_{tag}", tag="ps")
            nc.tensor.matmul(ms_ps[:], ones32[:], ssq[:])
            rstd = sm.tile([1, 1], F32, name=f"rstd_{tag}")
            nc.scalar.activation(rstd[:], ms_ps[:], AF.Sqrt,
                                 bias=eps11[:], scale=1.0 / HIDDEN)
            nc.vector.reciprocal(rstd[:], rstd[:])
            rstd_ps = pp.tile([32, 1], F32, name=f"rstdp_{tag}", tag="ps")
            nc.tensor.matmul(rstd_ps[:], ones_r32[:], rstd[:])
            rstd32 = sm.tile([32, 1], F32, name=f"rstd32_{tag}")
            nc.vector.tensor_copy(rstd32[:], rstd_ps[:])
            h_rows = sm.tile([32, 128], F32, name=f"hr_{tag}")
            nc.vector.tensor_tensor(h_rows[:], xr[:], nr[:], ALU.mult)
            nc.vector.tensor_scalar_mul(h_rows[:], h_rows[:], rstd32[:])
            h_ps = pp.tile([128, 32], F32, name=f"hps_{tag}", tag="ps")
            nc.tensor.transpose(h_ps[:], h_rows[:], ident32[:])
            h_cols = sm.tile([128, 32], F32, name=f"hc_{tag}")
            nc.vector.tensor_copy(h_cols[:], h_ps[:])
            hd = sm.tile([128, 32, 2], F16, name=f"hd_{tag}")
            nc.vector.tensor_copy(hd[:, :, 0], h_cols[:])
            tmp = sm.tile([128, 32], F32, name=f"htmp_{tag}")
            nc.vector.tensor_tensor(tmp[:], h_cols[:], hd[:, :, 0], ALU.subtract)
            nc.vector.tensor_copy(hd[:, :, 1], tmp[:])
            hs = sm.tile([128, 32], F16, name=f"hs_{tag}")
            nc.vector.tensor_scalar_mul(hs[:], h_cols[:], ILO)
            return hd, hs

        hd, hs = rmsnorm_dual(x_rows, anorm_rows, "a")

        # PE warm-up: the HAM clock gate releases (1.2 -> 2.4 GHz) only
        # after ~3.4us of sustained PE activity.  Burn dummy matmuls into a
        # scratch bank at points where the PE would otherwise sit idle
        # (kernel entry barrier, attention DVE phase, AllReduce waits) so
        # the real matvec streams run at full clock.
        warm_in = cpool.tile([128, 512], F16, name="warm_in")
        nc.vector.memset(warm_in[:], 0.0)

        def pe_warm(tag, count, stat):
            wps = pp.tile([2, 512], F32, name=f"warm_{tag}", tag="ps")
            for it in range(count):
                nc.tensor.matmul(wps[:], stat, warm_in[:],
                                 start=(it == 0), stop=(it == count - 1))
            sink = sm.tile([2, 1], F32, name=f"wsink_{tag}")
            nc.vector.tensor_copy(sink[:], wps[:, 0:1])

        if WARM:
            pe_warm("a", 20, hd[:, 0, :])

        # ---- q/k/v: one psum bank per projection (one accumulation group
        # per bank); lo accumulates on row 0, dual correction on row 1 ----
        qkv_ps = {w: pp.tile([2, QKV_N], F32, name=f"{w}_ps", tag="ps")
                  for w in ("wq", "wk", "wv")}
        for wi, w in enumerate(("wq", "wk", "wv")):
            ps = qkv_ps[w]
            kb0 = 0
            for t, rt in enumerate(QKV_RT):
                hi_t = hpool.tile([128, rt, QKV_N], F16, name="qkv_hi", tag="whi")
                nc.sync.dma_start(hi_t[:], i[w + "_hi"][:, kb0:kb0 + rt, :])
                lo_t = lpool.tile([128, rt, QKV_N], FP8, name="qkv_lo", tag="wlo")
                nc.sync.dma_start(lo_t[:], i[w + "_lo"][:, kb0:kb0 + rt, :])
                for b in range(rt):
                    kb = kb0 + b
                    nc.tensor.matmul(
                        ps[0:2, :], hd[:, kb, :], hi_t[:, b, :],
                        start=(kb == 0), stop=False,
                    )
                    nc.tensor.matmul(
                        ps[0:1, :], hs[:, kb:kb + 1], lo_t[:, b, :],
                        start=False, stop=(kb == KB - 1),
                    )
                kb0 += rt

        q_sb = sm.tile([2, QKV_N], F32, name="q_sb")
        nc.vector.tensor_copy(q_sb[:], qkv_ps["wq"][:])
        k_sb = sm.tile([2, QKV_N], F32, name="k_sb")
        nc.vector.tensor_copy(k_sb[:], qkv_ps["wk"][:])
        v16 = sm.tile([2, QKV_N], F16, name="v16")
        nc.vector.tensor_copy(v16[:], qkv_ps["wv"][:])

        # ---- RoPE (dual rows; q uses trig pre-scaled by 1/sqrt(d)) ----
        def rope(src, sin_t, cos_t, tag):
            out = sm.tile([2, QKV_N], F32, name=f"rope_{tag}")
            tmp = sm.tile([2, QKV_N], F32, name=f"ropetmp_{tag}")
            r3 = src[:].rearrange("p (h d) -> p h d", h=HEADS_PC)
            o3 = out[:].rearrange("p (h d) -> p h d", h=HEADS_PC)
            t3 = tmp[:].rearrange("p (h d) -> p h d", h=HEADS_PC)
            cb = cos_t[:].unsqueeze(1).to_broadcast((2, HEADS_PC, 64))
            sb = sin_t[:].unsqueeze(1).to_broadcast((2, HEADS_PC, 64))
            x1, x2 = r3[:, :, 0:64], r3[:, :, 64:128]
            nc.vector.tensor_tensor(o3[:, :, 0:64], x1, cb, ALU.mult)
            nc.vector.tensor_tensor(t3[:, :, 0:64], x2, sb, ALU.mult)
            nc.vector.tensor_sub(o3[:, :, 0:64], o3[:, :, 0:64],
                                 t3[:, :, 0:64])
            nc.vector.tensor_tensor(o3[:, :, 64:128], x2, cb, ALU.mult)
            nc.vector.tensor_tensor(t3[:, :, 64:128], x1, sb, ALU.mult)
            nc.vector.tensor_add(o3[:, :, 64:128], o3[:, :, 64:128],
                                 t3[:, :, 64:128])
            return out

        dbg("q_sb", q_sb[:], [2, QKV_N])
        dbg("k_sb", k_sb[:], [2, QKV_N])

        rope_q = rope(q_sb, trig["sinq"], trig["cosq"], "q")
        rope_k = rope(k_sb, trig["sink"], trig["cosk"], "k")
        dbg("rope_q", rope_q[:], [2, QKV_N])

        # q replicated to 128 partitions; the ones-matmul also sums the dual
        qrep_ps = pp.tile([128, QKV_N], F32, name="qrep_ps", tag="ps")
        nc.tensor.matmul(qrep_ps[:], ones2_128[:], rope_q[:])
        q_rep = sm.tile([128, QKV_N], F32, name="q_rep")
        nc.vector.tensor_copy(q_rep[:], qrep_ps[:])

        # current-token score: combine k dual via ones-matmul, then q.k
        kc_ps = pp.tile([1, QKV_N], F32, name="kc_ps", tag="ps")
        nc.tensor.matmul(kc_ps[:], ones2_1[:], rope_k[:])
        k_comb = sm.tile([1, QKV_N], F32, name="k_comb")
        nc.vector.tensor_copy(k_comb[:], kc_ps[:])
        prod_new = sm.tile([1, QKV_N], F32, name="prod_new")
        nc.vector.tensor_tensor(prod_new[:], k_comb[:], q_rep[0:1, :],
                                ALU.mult)
        s_new = sm.tile([1, HEADS_PC], F32, name="s_new")
        nc.vector.tensor_reduce(
            s_new[:], prod_new[:].rearrange("p (h d) -> p h d", h=HEADS_PC),
            mybir.AxisListType.X, ALU.add)
        e_new = sm.tile([1, HEADS_PC], F32, name="e_new")
        nc.scalar.activation(e_new[:], s_new[:], AF.Exp)
        e2_ps = pp.tile([2, HEADS_PC], F32, name="e2_ps", tag="ps")
        nc.tensor.matmul(e2_ps[:], ones1_2[:], e_new[:])
        e_new2 = sm.tile([2, HEADS_PC], F16, name="e_new2")
        nc.vector.tensor_copy(e_new2[:], e2_ps[:])

        # ---- attention over the KV cache ----
        o_ps = pp.tile([128, HEADS_PC], F32, name="o_ps", tag="ps")
        denom_acc = sm.tile([128, HEADS_PC], F32, name="denom_acc")
        nc.vector.memset(denom_acc[:], 0.0)

        for st in range(4):
            k_sup = kpool.tile([128, 8, QKV_N], F16, name="k_sup", tag="k")
            v_sup = vpool.tile([128, 8, QKV_N], F16, name="v_sup", tag="v")
            nc.sync.dma_start(k_sup[:], i["kc"][st])
            nc.sync.dma_start(v_sup[:], i["vc"][st])
            for b in range(8):
                scores = scr.tile([128, HEADS_PC], F32, name="scores", tag="sc")
                if USE_TTR:
                    prod = scr.tile([128, 128], F16, name="prod", tag="prod")
                    for h in range(HEADS_PC):
                        nc.vector.tensor_tensor_reduce(
                            prod[:], k_sup[:, b, h * 128:(h + 1) * 128],
                            q_rep[:, h * 128:(h + 1) * 128],
                            1.0, 0.0, ALU.mult, ALU.add, scores[:, h:h + 1])
                else:
                    prod = scr.tile([128, QKV_N], F16, name="prod", tag="prod")
                    nc.vector.tensor_tensor(prod[:], k_sup[:, b, :], q_rep[:],
                                            ALU.mult)
                    nc.vector.tensor_reduce(
                        scores[:],
                        prod[:].rearrange("p (h d) -> p h d", h=HEADS_PC),
                        mybir.AxisListType.X, ALU.add)
                expt = scr.tile([128, HEADS_PC], F32, name="expt", tag="ex")
                nc.scalar.activation(expt[:], scores[:], AF.Exp)
                nc.vector.tensor_add(denom_acc[:], denom_acc[:], expt[:])
                expt16 = scr.tile([128, HEADS_PC], F16, name="expt16", tag="e16")
                nc.scalar.copy(expt16[:], expt[:])
                for h in range(HEADS_PC):
                    nc.tensor.matmul(
                        o_ps[:, h:h + 1],
                        v_sup[:, b, h * 128:(h + 1) * 128],
                        expt16[:, h:h + 1],
                        start=(st == 0 and b == 0 and h == 0), stop=False,
                    )
        for h in range(HEADS_PC):
            nc.tensor.matmul(
                o_ps[:, h:h + 1], v16[:, h * 128:(h + 1) * 128],
                e_new2[:, h:h + 1],
                start=False, stop=(h == HEADS_PC - 1),
            )

        # denom = sum over tokens of the same fp16 expt + e_new
        d_ps = pp.tile([1, HEADS_PC], F32, name="d_ps", tag="ps")
        nc.tensor.matmul(d_ps[:], ones128[:], denom_acc[:])
        denom = sm.tile([1, HEADS_PC], F32, name="denom")
        nc.vector.tensor_copy(denom[:], d_ps[:])
        nc.vector.tensor_add(denom[:], denom[:], e_new[:])
        nc.vector.reciprocal(denom[:], denom[:])
        r_ps = pp.tile([128, HEADS_PC], F32, name="r_ps", tag="ps")
        nc.tensor.matmul(r_ps[:], ones_r128[:], denom[:])
        recip_bc = sm.tile([128, HEADS_PC], F32, name="recip_bc")
        nc.vector.tensor_copy(recip_bc[:], r_ps[:])
        o_sb = sm.tile([128, HEADS_PC], F32, name="o_sb")
        nc.vector.tensor_tensor(o_sb[:], o_ps[:], recip_bc[:], ALU.mult)
        dbg("denom", denom[:], [1, HEADS_PC])
        dbg("o_sb", o_sb[:], [128, HEADS_PC])

        od = sm.tile([128, HEADS_PC, 2], F16, name="od")
        nc.vector.tensor_copy(od[:, :, 0], o_sb[:])
        otmp = sm.tile([128, HEADS_PC], F32, name="otmp")
        nc.vector.tensor_tensor(otmp[:], o_sb[:], od[:, :, 0], ALU.subtract)
        nc.vector.tensor_copy(od[:, :, 1], otmp[:])
        os_ = sm.tile([128, HEADS_PC], F16, name="os_")
        nc.vector.tensor_scalar_mul(os_[:], o_sb[:], ILO)

        # ---- o @ w_o: 8 output chunks, one psum bank each ----
        if WARM:
            pe_warm("b", 10, od[:, 0, :])
        wo_ps = [pp.tile([2, 512], F32, name=f"wo_ps{n}", tag="ps")
                 for n in range(8)]
        r0 = 0
        for t, rt in enumerate(WO_RT):
            hi_t = hpool.tile([128, rt, HIDDEN], F16, name="wo_hi", tag="whi")
            nc.sync.dma_start(hi_t[:], i["wo_hi"][:, r0:r0 + rt, :])
            lo_t = lpool.tile([128, rt, HIDDEN], FP8, name="wo_lo", tag="wlo")
            nc.sync.dma_start(lo_t[:], i["wo_lo"][:, r0:r0 + rt, :])
            for b in range(rt):
                r = r0 + b
                for n in range(8):
                    nc.tensor.matmul(
                        wo_ps[n][0:2, :], od[:, r, :],
                        hi_t[:, b, 512 * n:512 * n + 512],
                        start=(r == 0), stop=False,
                    )
                    nc.tensor.matmul(
                        wo_ps[n][0:1, :], os_[:, r:r + 1],
                        lo_t[:, b, 512 * n:512 * n + 512],
                        start=False, stop=(r == HEADS_PC - 1),
                    )
            r0 += rt
        # stage as [2, 4096]: row 0 = hi+lo part, row 1 = dual correction;
        # the SWDGE pair below writes row 0 then accumulates row 1 on DRAM
        wo_sb = sm.tile([2, HIDDEN], F32, name="ar_stage", tag="ar_stage")
        for n in range(8):
            eng = nc.vector.tensor_copy if n % 2 == 0 else nc.scalar.copy
            eng(wo_sb[0:2, 512 * n:512 * n + 512], wo_ps[n][:])

        dbg("wo_sb", wo_sb[:], [2, HIDDEN])
        ar1_in = dram.tile([HIDDEN], F32, name="ar1_in")
        ar1_out = dram.tile([HIDDEN], F32, name="ar1_out")
        ar1v = ar1_in[:].rearrange("(a d) -> a d", a=1)
        nc.gpsimd.dma_start(ar1v, wo_sb[0:1, :])
        nc.gpsimd.dma_start(ar1v, wo_sb[1:2, :], accum_op=ALU.add)
        nc.gpsimd.collective_compute(
            "AllReduce", ALU.add,
            replica_groups=[list(range(N_CORES))],
            ins=[ar1_in[:].opt()], outs=[ar1_out[:].opt()],
        )

        # ---- MLP ----
        ar1_rows = sm.tile([32, 128], F32, name="ar1_rows")
        nc.sync.dma_start(ar1_rows[:], ar1_out[:].rearrange("(a d) -> a d", a=32))
        x2_rows = sm.tile([32, 128], F32, name="x2_rows")
        nc.vector.tensor_add(x2_rows[:], x_rows[:], ar1_rows[:])
        dbg("x2_rows", x2_rows[:], [32, 128])

        hd2, hs2 = rmsnorm_dual(x2_rows, fnorm_rows, "b")
        if WARM:
            pe_warm("c", 10, hd2[:, 0, :])

        f1_ps = [pp.tile([2, 512], F32, name=f"f1_ps{n}", tag="ps")
                 for n in range(3)]
        kb0 = 0
        for t, rt in enumerate(FF1_RT):
            hi_t = hpool.tile([128, 6, FF_NP], F16, name="f1_hi", tag="whi")
            lo_t = lpool.tile([128, 6, FF_NP], FP8, name="f1_lo", tag="wlo")
            nc.sync.dma_start(hi_t[:, 0:rt, :], i["wf1_hi"][:, kb0:kb0 + rt, :])
            nc.sync.dma_start(lo_t[:, 0:rt, :], i["wf1_lo"][:, kb0:kb0 + rt, :])
            for b in range(rt):
                kb = kb0 + b
                for n, (c0, w) in enumerate(FF1_CH):
                    nc.tensor.matmul(
                        f1_ps[n][0:2, 0:w], hd2[:, kb, :],
                        hi_t[:, b, c0:c0 + w],
                        start=(kb == 0), stop=False,
                    )
                    nc.tensor.matmul(
                        f1_ps[n][0:1, 0:w], hs2[:, kb:kb + 1],
                        lo_t[:, b, c0:c0 + w],
                        start=False, stop=(kb == KB - 1),
                    )
            kb0 += rt
        f1_sb = [sm.tile([2, 512], F32, name=f"f1_sb{n}") for n in range(3)]
        for n in range(3):
            nc.vector.tensor_copy(f1_sb[n][:], f1_ps[n][:])

        # a-columns via PE transposes of each dual pair (128-col windows)
        acol_ps = pp.tile([128, 22], F32, name="acol_ps", tag="ps")
        ident2 = ident32[0:2, 0:2]
        for j in range(11):
            n = (128 * j) // 512
            off = 128 * j - 512 * n
            nc.tensor.transpose(acol_ps[:, 2 * j:2 * j + 2],
                                f1_sb[n][:, off:off + 128], ident2)
        acol_sb = sm.tile([128, 22], F32, name="acol_sb")
        nc.vector.tensor_copy(acol_sb[:], acol_ps[:])
        pre = sm.tile([128, 11], F32, name="pre_silu")
        a3 = acol_sb[:].rearrange("p (j t) -> p j t", t=2)
        nc.vector.tensor_tensor(pre[:], a3[:, :, 0], a3[:, :, 1], ALU.add)
        sig = sm.tile([128, 11], F32, name="sig")
        nc.scalar.activation(sig[:], pre[:], AF.Sigmoid)
        a_sb = sm.tile([128, 11], F32, name="a_sb")
        nc.vector.tensor_tensor(a_sb[:], pre[:], sig[:], ALU.mult)
        dbg("a_sb", a_sb[:], [128, 11])
        ad = sm.tile([128, 11, 2], F16, name="ad")
        nc.vector.tensor_copy(ad[:, :, 0], a_sb[:])
        atmp = sm.tile([128, 11], F32, name="atmp")
        nc.vector.tensor_tensor(atmp[:], a_sb[:], ad[:, :, 0], ALU.subtract)
        nc.vector.tensor_copy(ad[:, :, 1], atmp[:])
        as_ = sm.tile([128, 11], F16, name="as_")
        nc.vector.tensor_scalar_mul(as_[:], a_sb[:], ILO)

        f2_ps = [pp.tile([2, 512], F32, name=f"f2_ps{n}", tag="ps")
                 for n in range(8)]
        kb0 = 0
        for t, rt in enumerate(FF2_RT):
            hi_t = hpool.tile([128, rt, HIDDEN], F16, name="f2_hi", tag="whi")
            nc.sync.dma_start(hi_t[:], i["wf2_hi"][:, kb0:kb0 + rt, :])
            lo_t = lpool.tile([128, rt, HIDDEN], FP8, name="f2_lo", tag="wlo")
            nc.sync.dma_start(lo_t[:], i["wf2_lo"][:, kb0:kb0 + rt, :])
            for b in range(rt):
                kb = kb0 + b
                for n in range(8):
                    nc.tensor.matmul(
                        f2_ps[n][0:2, :], ad[:, kb, :],
                        hi_t[:, b, 512 * n:512 * n + 512],
                        start=(kb == 0), stop=False,
                    )
                    nc.tensor.matmul(
                        f2_ps[n][0:1, :], as_[:, kb:kb + 1],
                        lo_t[:, b, 512 * n:512 * n + 512],
                        start=False, stop=(kb == 10),
                    )
            kb0 += rt
        ff_sb = sm.tile([2, HIDDEN], F32, name="ff_stage", tag="ar_stage")
        for n in range(8):
            eng = nc.vector.tensor_copy if n % 2 == 0 else nc.scalar.copy
            eng(ff_sb[0:2, 512 * n:512 * n + 512], f2_ps[n][:])

        ar2_in = dram.tile([HIDDEN], F32, name="ar2_in")
        ar2_out = dram.tile([HIDDEN], F32, name="ar2_out")
        ar2v = ar2_in[:].rearrange("(a d) -> a d", a=1)
        nc.gpsimd.dma_start(ar2v, ff_sb[0:1, :])
        nc.gpsimd.dma_start(ar2v, ff_sb[1:2, :], accum_op=ALU.add)
        nc.gpsimd.collective_compute(
            "AllReduce", ALU.add,
            replica_groups=[list(range(N_CORES))],
            ins=[ar2_in[:].opt()], outs=[ar2_out[:].opt()],
        )

        ar2_rows = sm.tile([32, 128], F32, name="ar2_rows")
        nc.sync.dma_start(ar2_rows[:], ar2_out[:].rearrange("(a d) -> a d", a=32))
        y_rows = sm.tile([32, 128], F32, name="y_rows")
        nc.vector.tensor_add(y_rows[:], x2_rows[:], ar2_rows[:])
        nc.sync.dma_start(y.rearrange("(a d) -> a d", a=32), y_rows[:])


_BUILT = None


def _build():
    global _BUILT
    if _BUILT is None:
        nc = bacc.Bacc("TRN2", target_bir_lowering=False, debug=False,
                       num_devices=N_CORES)
        with tile.TileContext(nc) as tc:
            _emit(nc, tc)
        nc.compile()
        _BUILT = nc
    return _BUILT


def _hilo(W):
    hi = W.astype(np.float16)
    res = (W - hi.astype(np.float32)) * LO
    lo = np.clip(res, -224.0, 224.0).astype(NP_FP8)
    return hi, lo


def _pack_rc(A, r128):
    """[r128*128, C] -> [128, r128, C] with row r*128+p on partition p."""
    C = A.shape[1]
    return np.ascontiguousarray(A.reshape(r128, 128, C).transpose(1, 0, 2))


def _shard(inputs):
    f = lambda a: np.ascontiguousarray(np.asarray(a, dtype=np.float32))
    x = f(inputs["x"])
    attn_norm = f(inputs["attn_norm"])
    ffn_norm = f(inputs["ffn_norm"])
    pos = int(np.asarray(inputs["pos"]))
    sin = f(inputs["sin_cache"][pos])
    cos = f(inputs["cos_cache"][pos])
    sinq = np.ascontiguousarray(np.stack([sin * SCALE] * 2).astype(np.float32))
    cosq = np.ascontiguousarray(np.stack([cos * SCALE] * 2).astype(np.float32))
    sink = np.ascontiguousarray(np.stack([sin] * 2).astype(np.float32))
    cosk = np.ascontiguousarray(np.stack([cos] * 2).astype(np.float32))
    wq, wk, wv = f(inputs["w_q"]), f(inputs["w_k"]), f(inputs["w_v"])
    wo, wf1, wf2 = f(inputs["w_o"]), f(inputs["w_ff1"]), f(inputs["w_ff2"])
    kc = f(inputs["k_cache"]).reshape(KV_LEN, N_HEADS * HEAD_DIM)
    vc = f(inputs["v_cache"]).reshape(KV_LEN, N_HEADS * HEAD_DIM)
    ident32 = np.eye(32, dtype=np.float32)

    in_maps = []
    for c in range(N_CORES):
        qs = slice(c * QKV_N, (c + 1) * QKV_N)
        fs = slice(c * FF_N, (c + 1) * FF_N)
        m = {
            "x": x, "attn_norm": attn_norm, "ffn_norm": ffn_norm,
            "sinq": sinq, "cosq": cosq, "sink": sink, "cosk": cosk,
            "ident32": ident32,
        }
        for name, wfull in (("wq", wq), ("wk", wk), ("wv", wv)):
            hi, lo = _hilo(wfull[:, qs])
            m[name + "_hi"] = _pack_rc(hi, KB)
            m[name + "_lo"] = _pack_rc(lo, KB)
        hi, lo = _hilo(wo[qs, :])
        m["wo_hi"] = _pack_rc(hi, HEADS_PC)
        m["wo_lo"] = _pack_rc(lo, HEADS_PC)
        w1 = np.pad(wf1[:, fs], ((0, 0), (0, FF_NP - FF_N)))
        hi, lo = _hilo(w1)
        m["wf1_hi"] = _pack_rc(hi, KB)
        m["wf1_lo"] = _pack_rc(lo, KB)
        w2 = np.pad(wf2[fs, :], ((0, FF_NP - FF_N), (0, 0)))
        hi, lo = _hilo(w2)
        m["wf2_hi"] = _pack_rc(hi, 11)
        m["wf2_lo"] = _pack_rc(lo, 11)
        # KV: [4096, 512] -> [4 supertiles, 128 part(=token%128), 8, 512]
        m["kc"] = np.ascontiguousarray(
            kc[:, qs].astype(np.float16).reshape(4, 8, 128, QKV_N)
            .transpose(0, 2, 1, 3))
        m["vc"] = np.ascontiguousarray(
            vc[:, qs].astype(np.float16).reshape(4, 8, 128, QKV_N)
            .transpose(0, 2, 1, 3))
        in_maps.append(m)
    return in_maps


def kernel(**inputs):
    nc = _build()
    in_maps = _shard(inputs)
    res = bass_utils.run_bass_kernel_spmd(
        nc, in_maps, core_ids=list(range(N_CORES)))
    return res.results[0]["y"]


# revision 50
# speedup vs baseline: 1.1304x; 1.1304x over previous
"""Llama decode block (single token) on 8 TRN2 NeuronCores, tensor-parallel.

Sharding (per sharding_hint): w_q/w_k/w_v/w_ff1 column-sharded, w_o/w_ff2
row-sharded, KV cache sharded by head (4 heads/core). AllReduce after the
attention output projection and after w_ff2. Residuals (x, x2) are added
locally after each AllReduce, so the AR payloads carry only matvec partials.

Memory-bound problem -> minimize HBM bytes and DMA descriptor count:

* Every weight is split on the host into a float16 "hi" part plus a
  float8e4 "lo" part holding 2^12*(W - hi).  3 bytes/elem instead of 4,
  with ~1e-5 effective relative error (fp16 residual quantized by fp8).
* All tensors are pre-packed on the host into the exact SBUF tile layout
  ([128 partitions, r, cols], row g(p,r)=r*128+p), so every DMA moves
  fully contiguous 16-32KB per-partition lines (the f32 baseline was
  descriptor-rate-bound at ~181 GB/s on 2KB strided descriptors).
* KV cache is fp16 (fp16 scores/attn keeps max rel err ~6e-3 vs the 2e-2
  gate; bf16 fails it).

Matvec scheme (per k-block): activation h is kept as an fp16 dual
(h1=fp16(h), h2=fp16(h-h1)) -> matmul with 2 stationary columns gives
psum rows [h1@Whi ; h2@Whi] at no extra PE cost; the lo pass uses
hs=fp16(h*2^-12) against Wlo=fp8(2^12*residual) and accumulates onto row
0, so products come out correctly scaled with no fixup pass. Row pairs
are combined either by the next matmul's contraction (q replicate, v
new-token), by PE transposes (ff1 -> silu columns), or by a SWDGE
accumulate DMA when writing the AllReduce input (wo, ff2).
"""

import math

import numpy as np
import ml_dtypes

import concourse.bass as bass
import concourse.mybir as mybir
import concourse.tile as tile
from concourse import bacc
from concourse import bass_utils

F32 = mybir.dt.float32
F16 = mybir.dt.float16
FP8 = mybir.dt.float8e4
AF = mybir.ActivationFunctionType
ALU = mybir.AluOpType

HIDDEN = 4096
N_HEADS = 32
HEAD_DIM = 128
INTERM = 11008
KV_LEN = 4096
N_CORES = 8

HEADS_PC = N_HEADS // N_CORES          # 4 heads per core
QKV_N = HEADS_PC * HEAD_DIM            # 512
FF_N = INTERM // N_CORES               # 1376
FF_NP = 1408                           # padded to 11*128
KB = HIDDEN // 128                     # 32 k-blocks of the hidden dim
SCALE = 1.0 / math.sqrt(HEAD_DIM)
LO = 2.0 ** 12                         # residual scale for the fp8 stream
ILO = 1.0 / LO

NP_FP8 = ml_dtypes.float8_e4m3

DEBUG = False          # emit intermediate tensors as extra outputs
WARM = False           # PE warm-up bursts
USE_TTR = False        # fused multiply-reduce for attention scores

# weight-stream tiling (r-blocks per DMA tile)
QKV_RT = [16, 16]                      # 32 kb in 2 tiles of [128,16,512]
WO_RT = [2, 2]                         # 4 r in 2 tiles of [128,2,4096]
FF1_RT = [6, 6, 6, 6, 6, 2]            # 32 kb over [128,r,1408] tiles
FF2_RT = [2, 2, 2, 2, 2, 1]            # 11 kb over [128,r,4096] tiles
FF1_CH = [(0, 512), (512, 512), (1024, 384)]   # ff1 psum column chunks


def _emit(nc, tc):
    i = {}

    def din(name, shape, dt=F32):
        i[name] = nc.dram_tensor(name, list(shape), dt, kind="ExternalInput").ap()

    din("x", [HIDDEN])
    din("attn_norm", [HIDDEN])
    din("ffn_norm", [HIDDEN])
    din("sinq", [2, 64])               # pre-scaled by 1/sqrt(d)
    din("cosq", [2, 64])
    din("sink", [2, 64])
    din("cosk", [2, 64])
    din("ident32", [32, 32])
    for w in ("wq", "wk", "wv"):
        din(w + "_hi", [128, KB, QKV_N], F16)
        din(w + "_lo", [128, KB, QKV_N], FP8)
    din("wo_hi", [128, HEADS_PC, HIDDEN], F16)
    din("wo_lo", [128, HEADS_PC, HIDDEN], FP8)
    din("wf1_hi", [128, KB, FF_NP], F16)
    din("wf1_lo", [128, KB, FF_NP], FP8)
    din("wf2_hi", [128, 11, HIDDEN], F16)
    din("wf2_lo", [128, 11, HIDDEN], FP8)
    din("kc", [4, 128, 8, QKV_N], F16)
    din("vc", [4, 128, 8, QKV_N], F16)
    y = nc.dram_tensor("y", [HIDDEN], F32, kind="ExternalOutput").ap()

    dbg_outs = {}

    def dbg(name, src_ap, shape):
        if not DEBUG:
            return
        d = nc.dram_tensor("dbg_" + name, list(shape), F32,
                           kind="ExternalOutput").ap()
        nc.sync.dma_start(d, src_ap)
        dbg_outs[name] = d

    with (
        tc.tile_pool(name="const", bufs=1) as cpool,
        tc.tile_pool(name="whi", bufs=5) as hpool,
        tc.tile_pool(name="wlo", bufs=4) as lpool,
        tc.tile_pool(name="kpool", bufs=2) as kpool,
        tc.tile_pool(name="vpool", bufs=2) as vpool,
        tc.tile_pool(name="sm", bufs=1) as sm,
        tc.tile_pool(name="scr", bufs=2) as scr,
        tc.tile_pool(name="psum", bufs=8, space="PSUM") as pp,
        tc.tile_pool(name="dram", bufs=1, space="DRAM") as dram,
    ):
        # ---- constants ----
        ones32 = cpool.tile([32, 1], F32)
        nc.vector.memset(ones32[:], 1.0)
        ones128 = cpool.tile([128, 1], F32)
        nc.vector.memset(ones128[:], 1.0)
        ones_r32 = cpool.tile([1, 32], F32)
        nc.vector.memset(ones_r32[:], 1.0)
        ones_r128 = cpool.tile([1, 128], F32)
        nc.vector.memset(ones_r128[:], 1.0)
        ones2_128 = cpool.tile([2, 128], F32)
        nc.vector.memset(ones2_128[:], 1.0)
        ones2_1 = cpool.tile([2, 1], F32)
        nc.vector.memset(ones2_1[:], 1.0)
        ones1_2 = cpool.tile([1, 2], F32)
        nc.vector.memset(ones1_2[:], 1.0)
        eps11 = cpool.tile([1, 1], F32)
        nc.vector.memset(eps11[:], 1e-6)
        ident32 = cpool.tile([32, 32], F32)
        nc.sync.dma_start(ident32[:], i["ident32"])
        trig = {}
        for t in ("sinq", "cosq", "sink", "cosk"):
            trig[t] = cpool.tile([2, 64], F32, name=t)
            nc.sync.dma_start(trig[t][:], i[t])

        x_rows = cpool.tile([32, 128], F32)
        nc.sync.dma_start(x_rows[:], i["x"].rearrange("(a d) -> a d", a=32))
        anorm_rows = cpool.tile([32, 128], F32)
        nc.sync.dma_start(anorm_rows[:],
                          i["attn_norm"].rearrange("(a d) -> a d", a=32))
        fnorm_rows = cpool.tile([32, 128], F32)
        nc.sync.dma_start(fnorm_rows[:],
                          i["ffn_norm"].rearrange("(a d) -> a d", a=32))

        # ---- rmsnorm -> fp16 dual columns hd[128,32,2], hs[128,32] ----
        def rmsnorm_dual(xr, nr, tag):
            sq = sm.tile([32, 128], F32, name=f"sq_{tag}")
            ssq = sm.tile([32, 1], F32, name=f"ssq_{tag}")
            nc.scalar.activation(sq[:], xr[:], AF.Square, accum_out=ssq[:])
            ms_ps = pp.tile([1, 1], F32, name=f"ms_{tag}", tag="ps")
            nc.tensor.matmul(ms_ps[:], ones32[:], ssq[:])
            rstd = sm.tile([1, 1], F32, name=f"rstd_{tag}")
            nc.scalar.activation(rstd[:], ms_ps[:], AF.Sqrt,
                                 bias=eps11[:], scale=1.0 / HIDDEN)
            nc.vector.reciprocal(rstd[:], rstd[:])
            rstd_ps = pp.tile([32, 1], F32, name=f"rstdp_{tag}", tag="ps")
            nc.tensor.matmul(rstd_ps[:], ones_r32[:], rstd[:])
            rstd32 = sm.tile([32, 1], F32, name=f"rstd32_{tag}")
            nc.vector.tensor_copy(rstd32[:], rstd_ps[:])
            h_rows = sm.tile([32, 128], F32, name=f"hr_{tag}")
            nc.vector.tensor_tensor(h_rows[:], xr[:], nr[:], ALU.mult)
            nc.vector.tensor_scalar_mul(h_rows[:], h_rows[:], rstd32[:])
            h_ps = pp.tile([128, 32], F32, name=f"hps_{tag}", tag="ps")
            nc.tensor.transpose(h_ps[:], h_rows[:], ident32[:])
            h_cols = sm.tile([128, 32], F32, name=f"hc_{tag}")
            nc.vector.tensor_copy(h_cols[:], h_ps[:])
            hd = sm.tile([128, 32, 2], F16, name=f"hd_{tag}")
            nc.vector.tensor_copy(hd[:, :, 0], h_cols[:])
            tmp = sm.tile([128, 32], F32, name=f"htmp_{tag}")
            nc.vector.tensor_tensor(tmp[:], h_cols[:], hd[:, :, 0], ALU.subtract)
            nc.vector.tensor_copy(hd[:, :, 1], tmp[:])
            hs = sm.tile([128, 32], F16, name=f"hs_{tag}")
            nc.vector.tensor_scalar_mul(hs[:], h_cols[:], ILO)
            return hd, hs

        hd, hs = rmsnorm_dual(x_rows, anorm_rows, "a")

        # PE warm-up: the HAM clock gate releases (1.2 -> 2.4 GHz) only
        # after ~3.4us of sustained PE activity.  Burn dummy matmuls into a
        # scratch bank at points where the PE would otherwise sit idle
        # (kernel entry barrier, attention DVE phase, AllReduce waits) so
        # the real matvec streams run at full clock.
        warm_in = cpool.tile([128, 512], F16, name="warm_in")
        nc.vector.memset(warm_in[:], 0.0)

        def pe_warm(tag, count, stat):
            wps = pp.tile([2, 512], F32, name=f"warm_{tag}", tag="ps")
            for it in range(count):
                nc.tensor.matmul(wps[:], stat, warm_in[:],
                                 start=(it == 0), stop=(it == count - 1))
            sink = sm.tile([2, 1], F32, name=f"wsink_{tag}")
            nc.vector.tensor_copy(sink[:], wps[:, 0:1])

        if WARM:
            pe_warm("a", 20, hd[:, 0, :])

        # ---- q/k/v: one psum bank per projection (one accumulation group
        # per bank); lo accumulates on row 0, dual correction on row 1 ----
        qkv_ps = {w: pp.tile([2, QKV_N], F32, name=f"{w}_ps", tag="ps")
                  for w in ("wq", "wk", "wv")}
        for wi, w in enumerate(("wq", "wk", "wv")):
            ps = qkv_ps[w]
            kb0 = 0
            for t, rt in enumerate(QKV_RT):
                hi_t = hpool.tile([128, rt, QKV_N], F16, name="qkv_hi", tag="whi")
                nc.sync.dma_start(hi_t[:], i[w + "_hi"][:, kb0:kb0 + rt, :])
                lo_t = lpool.tile([128, rt, QKV_N], FP8, name="qkv_lo", tag="wlo")
                nc.sync.dma_start(lo_t[:], i[w + "_lo"][:, kb0:kb0 + rt, :])
                for b in range(rt):
                    kb = kb0 + b
                    nc.tensor.matmul(
                        ps[0:2, :], hd[:, kb, :], hi_t[:, b, :],
                        start=(kb == 0), stop=False,
                    )
                    nc.tensor.matmul(
                        ps[0:1, :], hs[:, kb:kb + 1], lo_t[:, b, :],
                        start=False, stop=(kb == KB - 1),
                    )
                kb0 += rt

        q_sb = sm.tile([2, QKV_N], F32, name="q_sb")
        nc.vector.tensor_copy(q_sb[:], qkv_ps["wq"][:])
        k_sb = sm.tile([2, QKV_N], F32, name="k_sb")
        nc.vector.tensor_copy(k_sb[:], qkv_ps["wk"][:])
        v16 = sm.tile([2, QKV_N], F16, name="v16")
        nc.vector.tensor_copy(v16[:], qkv_ps["wv"][:])

        # ---- RoPE (dual rows; q uses trig pre-scaled by 1/sqrt(d)) ----
        def rope(src, sin_t, cos_t, tag):
            out = sm.tile([2, QKV_N], F32, name=f"rope_{tag}")
            tmp = sm.tile([2, QKV_N], F32, name=f"ropetmp_{tag}")
            r3 = src[:].rearrange("p (h d) -> p h d", h=HEADS_PC)
            o3 = out[:].rearrange("p (h d) -> p h d", h=HEADS_PC)
            t3 = tmp[:].rearrange("p (h d) -> p h d", h=HEADS_PC)
            cb = cos_t[:].unsqueeze(1).to_broadcast((2, HEADS_PC, 64))
            sb = sin_t[:].unsqueeze(1).to_broadcast((2, HEADS_PC, 64))
            x1, x2 = r3[:, :, 0:64], r3[:, :, 64:128]
            nc.vector.tensor_tensor(o3[:, :, 0:64], x1, cb, ALU.mult)
            nc.vector.tensor_tensor(t3[:, :, 0:64], x2, sb, ALU.mult)
            nc.vector.tensor_sub(o3[:, :, 0:64], o3[:, :, 0:64],
                                 t3[:, :, 0:64])
            nc.vector.tensor_tensor(o3[:, :, 64:128], x2, cb, ALU.mult)
            nc.vector.tensor_tensor(t3[:, :, 64:128], x1, sb, ALU.mult)
            nc.vector.tensor_add(o3[:, :, 64:128], o3[:, :, 64:128],
                                 t3[:, :, 64:128])
            return out

        dbg("q_sb", q_sb[:], [2, QKV_N])
        dbg("k_sb", k_sb[:], [2, QKV_N])

        rope_q = rope(q_sb, trig["sinq"], trig["cosq"], "q")
        rope_k = rope(k_sb, trig["sink"], trig["cosk"], "k")
        dbg("rope_q", rope_q[:], [2, QKV_N])

        # q replicated to 128 partitions; the ones-matmul also sums the dual
        qrep_ps = pp.tile([128, QKV_N], F32, name="qrep_ps", tag="ps")
        nc.tensor.matmul(qrep_ps[:], ones2_128[:], rope_q[:])
        q_rep = sm.tile([128, QKV_N], F16, name="q_rep")
        nc.vector.tensor_copy(q_rep[:], qrep_ps[:])
        q_rep32 = sm.tile([2, QKV_N], F32, name="q_rep32")
        nc.vector.tensor_copy(q_rep32[:], qrep_ps[0:2, :])

        # current-token score: combine k dual via ones-matmul, then q.k
        kc_ps = pp.tile([1, QKV_N], F32, name="kc_ps", tag="ps")
        nc.tensor.matmul(kc_ps[:], ones2_1[:], rope_k[:])
        k_comb = sm.tile([1, QKV_N], F32, name="k_comb")
        nc.vector.tensor_copy(k_comb[:], kc_ps[:])
        prod_new = sm.tile([1, QKV_N], F32, name="prod_new")
        nc.vector.tensor_tensor(prod_new[:], k_comb[:], q_rep32[0:1, :],
                                ALU.mult)
        s_new = sm.tile([1, HEADS_PC], F32, name="s_new")
        nc.vector.tensor_reduce(
            s_new[:], prod_new[:].rearrange("p (h d) -> p h d", h=HEADS_PC),
            mybir.AxisListType.X, ALU.add)
        e_new = sm.tile([1, HEADS_PC], F32, name="e_new")
        nc.scalar.activation(e_new[:], s_new[:], AF.Exp)
        e2_ps = pp.tile([2, HEADS_PC], F32, name="e2_ps", tag="ps")
        nc.tensor.matmul(e2_ps[:], ones1_2[:], e_new[:])
        e_new2 = sm.tile([2, HEADS_PC], F16, name="e_new2")
        nc.vector.tensor_copy(e_new2[:], e2_ps[:])

        # ---- attention over the KV cache ----
        o_ps = pp.tile([128, HEADS_PC], F32, name="o_ps", tag="ps")
        denom_acc = sm.tile([128, HEADS_PC], F32, name="denom_acc")
        nc.vector.memset(denom_acc[:], 0.0)

        for st in range(4):
            k_sup = kpool.tile([128, 8, QKV_N], F16, name="k_sup", tag="k")
            v_sup = vpool.tile([128, 8, QKV_N], F16, name="v_sup", tag="v")
            nc.sync.dma_start(k_sup[:], i["kc"][st])
            nc.sync.dma_start(v_sup[:], i["vc"][st])
            for b in range(8):
                scores = scr.tile([128, HEADS_PC], F32, name="scores", tag="sc")
                if USE_TTR:
                    prod = scr.tile([128, 128], F16, name="prod", tag="prod")
                    for h in range(HEADS_PC):
                        nc.vector.tensor_tensor_reduce(
                            prod[:], k_sup[:, b, h * 128:(h + 1) * 128],
                            q_rep[:, h * 128:(h + 1) * 128],
                            1.0, 0.0, ALU.mult, ALU.add, scores[:, h:h + 1])
                else:
                    prod = scr.tile([128, QKV_N], F16, name="prod", tag="prod")
                    nc.vector.tensor_tensor(prod[:], k_sup[:, b, :], q_rep[:],
                                            ALU.mult)
                    nc.vector.tensor_reduce(
                        scores[:],
                        prod[:].rearrange("p (h d) -> p h d", h=HEADS_PC),
                        mybir.AxisListType.X, ALU.add)
                expt = scr.tile([128, HEADS_PC], F32, name="expt", tag="ex")
                nc.scalar.activation(expt[:], scores[:], AF.Exp)
                nc.vector.tensor_add(denom_acc[:], denom_acc[:], expt[:])
                expt16 = scr.tile([128, HEADS_PC], F16, name="expt16", tag="e16")
                nc.scalar.copy(expt16[:], expt[:])
                for h in range(HEADS_PC):
                    nc.tensor.matmul(
                        o_ps[:, h:h + 1],
                        v_sup[:, b, h * 128:(h + 1) * 128],
                        expt16[:, h:h + 1],
                        start=(st == 0 and b == 0 and h == 0), stop=False,
                    )
        for h in range(HEADS_PC):
            nc.tensor.matmul(
                o_ps[:, h:h + 1], v16[:, h * 128:(h + 1) * 128],
                e_new2[:, h:h + 1],
                start=False, stop=(h == HEADS_PC - 1),
            )

        # denom = sum over tokens of the same fp16 expt + e_new
        d_ps = pp.tile([1, HEADS_PC], F32, name="d_ps", tag="ps")
        nc.tensor.matmul(d_ps[:], ones128[:], denom_acc[:])
        denom = sm.tile([1, HEADS_PC], F32, name="denom")
        nc.vector.tensor_copy(denom[:], d_ps[:])
        nc.vector.tensor_add(denom[:], denom[:], e_new[:])
        nc.vector.reciprocal(denom[:], denom[:])
        r_ps = pp.tile([128, HEADS_PC], F32, name="r_ps", tag="ps")
        nc.tensor.matmul(r_ps[:], ones_r128[:], denom[:])
        recip_bc = sm.tile([128, HEADS_PC], F32, name="recip_bc")
        nc.vector.tensor_copy(recip_bc[:], r_ps[:])
        o_sb = sm.tile([128, HEADS_PC], F32, name="o_sb")
        nc.vector.tensor_tensor(o_sb[:], o_ps[:], recip_bc[:], ALU.mult)
        dbg("denom", denom[:], [1, HEADS_PC])
        dbg("o_sb", o_sb[:], [128, HEADS_PC])

        od = sm.tile([128, HEADS_PC, 2], F16, name="od")
        nc.vector.tensor_copy(od[:, :, 0], o_sb[:])
        otmp = sm.tile([128, HEADS_PC], F32, name="otmp")
        nc.vector.tensor_tensor(otmp[:], o_sb[:], od[:, :, 0], ALU.subtract)
        nc.vector.tensor_copy(od[:, :, 1], otmp[:])
        os_ = sm.tile([128, HEADS_PC], F16, name="os_")
        nc.vector.tensor_scalar_mul(os_[:], o_sb[:], ILO)

        # ---- o @ w_o: 8 output chunks, one psum bank each ----
        if WARM:
            pe_warm("b", 10, od[:, 0, :])
        wo_ps = [pp.tile([2, 512], F32, name=f"wo_ps{n}", tag="ps")
                 for n in range(8)]
        r0 = 0
        for t, rt in enumerate(WO_RT):
            hi_t = hpool.tile([128, rt, HIDDEN], F16, name="wo_hi", tag="whi")
            nc.sync.dma_start(hi_t[:], i["wo_hi"][:, r0:r0 + rt, :])
            lo_t = lpool.tile([128, rt, HIDDEN], FP8, name="wo_lo", tag="wlo")
            nc.sync.dma_start(lo_t[:], i["wo_lo"][:, r0:r0 + rt, :])
            for b in range(rt):
                r = r0 + b
                for n in range(8):
                    nc.tensor.matmul(
                        wo_ps[n][0:2, :], od[:, r, :],
                        hi_t[:, b, 512 * n:512 * n + 512],
                        start=(r == 0), stop=False,
                    )
                    nc.tensor.matmul(
                        wo_ps[n][0:1, :], os_[:, r:r + 1],
                        lo_t[:, b, 512 * n:512 * n + 512],
                        start=False, stop=(r == HEADS_PC - 1),
                    )
            r0 += rt
        # stage as [2, 4096]: row 0 = hi+lo part, row 1 = dual correction;
        # the SWDGE pair below writes row 0 then accumulates row 1 on DRAM
        wo_sb = sm.tile([2, HIDDEN], F32, name="ar_stage", tag="ar_stage")
        for n in range(8):
            eng = nc.vector.tensor_copy if n % 2 == 0 else nc.scalar.copy
            eng(wo_sb[0:2, 512 * n:512 * n + 512], wo_ps[n][:])

        dbg("wo_sb", wo_sb[:], [2, HIDDEN])
        ar1_in = dram.tile([HIDDEN], F32, name="ar1_in")
        ar1_out = dram.tile([HIDDEN], F32, name="ar1_out")
        ar1v = ar1_in[:].rearrange("(a d) -> a d", a=1)
        nc.gpsimd.dma_start(ar1v, wo_sb[0:1, :])
        nc.gpsimd.dma_start(ar1v, wo_sb[1:2, :], accum_op=ALU.add)
        nc.gpsimd.collective_compute(
            "AllReduce", ALU.add,
            replica_groups=[list(range(N_CORES))],
            ins=[ar1_in[:].opt()], outs=[ar1_out[:].opt()],
        )

        # ---- MLP ----
        ar1_rows = sm.tile([32, 128], F32, name="ar1_rows")
        nc.sync.dma_start(ar1_rows[:], ar1_out[:].rearrange("(a d) -> a d", a=32))
        x2_rows = sm.tile([32, 128], F32, name="x2_rows")
        nc.vector.tensor_add(x2_rows[:], x_rows[:], ar1_rows[:])
        dbg("x2_rows", x2_rows[:], [32, 128])

        hd2, hs2 = rmsnorm_dual(x2_rows, fnorm_rows, "b")
        if WARM:
            pe_warm("c", 10, hd2[:, 0, :])

        f1_ps = [pp.tile([2, 512], F32, name=f"f1_ps{n}", tag="ps")
                 for n in range(3)]
        kb0 = 0
        for t, rt in enumerate(FF1_RT):
            hi_t = hpool.tile([128, 6, FF_NP], F16, name="f1_hi", tag="whi")
            lo_t = lpool.tile([128, 6, FF_NP], FP8, name="f1_lo", tag="wlo")
            nc.sync.dma_start(hi_t[:, 0:rt, :], i["wf1_hi"][:, kb0:kb0 + rt, :])
            nc.sync.dma_start(lo_t[:, 0:rt, :], i["wf1_lo"][:, kb0:kb0 + rt, :])
            for b in range(rt):
                kb = kb0 + b
                for n, (c0, w) in enumerate(FF1_CH):
                    nc.tensor.matmul(
                        f1_ps[n][0:2, 0:w], hd2[:, kb, :],
                        hi_t[:, b, c0:c0 + w],
                        start=(kb == 0), stop=False,
                    )
                    nc.tensor.matmul(
                        f1_ps[n][0:1, 0:w], hs2[:, kb:kb + 1],
                        lo_t[:, b, c0:c0 + w],
                        start=False, stop=(kb == KB - 1),
                    )
            kb0 += rt
        f1_sb = [sm.tile([2, 512], F32, name=f"f1_sb{n}") for n in range(3)]
        for n in range(3):
            nc.vector.tensor_copy(f1_sb[n][:], f1_ps[n][:])

        # a-columns via PE transposes of each dual pair (128-col windows)
        acol_ps = pp.tile([128, 22], F32, name="acol_ps", tag="ps")
        ident2 = ident32[0:2, 0:2]
        for j in range(11):
            n = (128 * j) // 512
            off = 128 * j - 512 * n
            nc.tensor.transpose(acol_ps[:, 2 * j:2 * j + 2],
                                f1_sb[n][:, off:off + 128], ident2)
        acol_sb = sm.tile([128, 22], F32, name="acol_sb")
        nc.vector.tensor_copy(acol_sb[:], acol_ps[:])
        pre = sm.tile([128, 11], F32, name="pre_silu")
        a3 = acol_sb[:].rearrange("p (j t) -> p j t", t=2)
        nc.vector.tensor_tensor(pre[:], a3[:, :, 0], a3[:, :, 1], ALU.add)
        sig = sm.tile([128, 11], F32, name="sig")
        nc.scalar.activation(sig[:], pre[:], AF.Sigmoid)
        a_sb = sm.tile([128, 11], F32, name="a_sb")
        nc.vector.tensor_tensor(a_sb[:], pre[:], sig[:], ALU.mult)
        dbg("a_sb", a_sb[:], [128, 11])
        ad = sm.tile([128, 11, 2], F16, name="ad")
        nc.vector.tensor_copy(ad[:, :, 0], a_sb[:])
        atmp = sm.tile([128, 11], F32, name="atmp")
        nc.vector.tensor_tensor(atmp[:], a_sb[:], ad[:, :, 0], ALU.subtract)
        nc.vector.tensor_copy(ad[:, :, 1], atmp[:])
        as_ = sm.tile([128, 11], F16, name="as_")
        nc.vector.tensor_scalar_mul(as_[:], a_sb[:], ILO)

        f2_ps = [pp.tile([2, 512], F32, name=f"f2_ps{n}", tag="ps")
                 for n in range(8)]
        kb0 = 0
        for t, rt in enumerate(FF2_RT):
            hi_t = hpool.tile([128, rt, HIDDEN], F16, name="f2_hi", tag="whi")
            nc.sync.dma_start(hi_t[:], i["wf2_hi"][:, kb0:kb0 + rt, :])
            lo_t = lpool.tile([128, rt, HIDDEN], FP8, name="f2_lo", tag="wlo")
            nc.sync.dma_start(lo_t[:], i["wf2_lo"][:, kb0:kb0 + rt, :])
            for b in range(rt):
                kb = kb0 + b
                for n in range(8):
                    nc.tensor.matmul(
                        f2_ps[n][0:2, :], ad[:, kb, :],
                        hi_t[:, b, 512 * n:512 * n + 512],
                        start=(kb == 0), stop=False,
                    )
                    nc.tensor.matmul(
                        f2_ps[n][0:1, :], as_[:, kb:kb + 1],
                        lo_t[:, b, 512 * n:512 * n + 512],
                        start=False, stop=(kb == 10),
                    )
            kb0 += rt
        ff_sb = sm.tile([2, HIDDEN], F32, name="ff_stage", tag="ar_stage")
        for n in range(8):
            eng = nc.vector.tensor_copy if n % 2 == 0 else nc.scalar.copy
            eng(ff_sb[0:2, 512 * n:512 * n + 512], f2_ps[n][:])

        ar2_in = dram.tile([HIDDEN], F32, name="ar2_in")
        ar2_out = dram.tile([HIDDEN], F32, name="ar2_out")
        ar2v = ar2_in[:].rearrange("(a d) -> a d", a=1)
        nc.gpsimd.dma_start(ar2v, ff_sb[0:1, :])
        nc.gpsimd.dma_start(ar2v, ff_sb[1:2, :], accum_op=ALU.add)
        nc.gpsimd.collective_compute(
            "AllReduce", ALU.add,
            replica_groups=[list(range(N_CORES))],
            ins=[ar2_in[:].opt()], outs=[ar2_out[:].opt()],
        )

        ar2_rows = sm.tile([32, 128], F32, name="ar2_rows")
        nc.sync.dma_start(ar2_rows[:], ar2_out[:].rearrange("(a d) -> a d", a=32))
        y_rows = sm.tile([32, 128], F32, name="y_rows")
        nc.vector.tensor_add(y_rows[:], x2_rows[:], ar2_rows[:])
        nc.sync.dma_start(y.rearrange("(a d) -> a d", a=32), y_rows[:])


_BUILT = None


def _build():
    global _BUILT
    if _BUILT is None:
        nc = bacc.Bacc("TRN2", target_bir_lowering=False, debug=False,
                       num_devices=N_CORES)
        with tile.TileContext(nc) as tc:
            _emit(nc, tc)
        nc.compile()
        _BUILT = nc
    return _BUILT


def _hilo(W):
    hi = W.astype(np.float16)
    res = (W - hi.astype(np.float32)) * LO
    lo = np.clip(res, -224.0, 224.0).astype(NP_FP8)
    return hi, lo


def _pack_rc(A, r128):
    """[r128*128, C] -> [128, r128, C] with row r*128+p on partition p."""
    C = A.shape[1]
    return np.ascontiguousarray(A.reshape(r128, 128, C).transpose(1, 0, 2))


def _shard(inputs):
    f = lambda a: np.ascontiguousarray(np.asarray(a, dtype=np.float32))
    x = f(inputs["x"])
    attn_norm = f(inputs["attn_norm"])
    ffn_norm = f(inputs["ffn_norm"])
    pos = int(np.asarray(inputs["pos"]))
    sin = f(inputs["sin_cache"][pos])
    cos = f(inputs["cos_cache"][pos])
    sinq = np.ascontiguousarray(np.stack([sin * SCALE] * 2).astype(np.float32))
    cosq = np.ascontiguousarray(np.stack([cos * SCALE] * 2).astype(np.float32))
    sink = np.ascontiguousarray(np.stack([sin] * 2).astype(np.float32))
    cosk = np.ascontiguousarray(np.stack([cos] * 2).astype(np.float32))
    wq, wk, wv = f(inputs["w_q"]), f(inputs["w_k"]), f(inputs["w_v"])
    wo, wf1, wf2 = f(inputs["w_o"]), f(inputs["w_ff1"]), f(inputs["w_ff2"])
    kc = f(inputs["k_cache"]).reshape(KV_LEN, N_HEADS * HEAD_DIM)
    vc = f(inputs["v_cache"]).reshape(KV_LEN, N_HEADS * HEAD_DIM)
    ident32 = np.eye(32, dtype=np.float32)

    in_maps = []
    for c in range(N_CORES):
        qs = slice(c * QKV_N, (c + 1) * QKV_N)
        fs = slice(c * FF_N, (c + 1) * FF_N)
        m = {
            "x": x, "attn_norm": attn_norm, "ffn_norm": ffn_norm,
            "sinq": sinq, "cosq": cosq, "sink": sink, "cosk": cosk,
            "ident32": ident32,
        }
        for name, wfull in (("wq", wq), ("wk", wk), ("wv", wv)):
            hi, lo = _hilo(wfull[:, qs])
            m[name + "_hi"] = _pack_rc(hi, KB)
            m[name + "_lo"] = _pack_rc(lo, KB)
        hi, lo = _hilo(wo[qs, :])
        m["wo_hi"] = _pack_rc(hi, HEADS_PC)
        m["wo_lo"] = _pack_rc(lo, HEADS_PC)
        w1 = np.pad(wf1[:, fs], ((0, 0), (0, FF_NP - FF_N)))
        hi, lo = _hilo(w1)
        m["wf1_hi"] = _pack_rc(hi, KB)
        m["wf1_lo"] = _pack_rc(lo, KB)
        w2 = np.pad(wf2[fs, :], ((0, FF_NP - FF_N), (0, 0)))
        hi, lo = _hilo(w2)
        m["wf2_hi"] = _pack_rc(hi, 11)
        m["wf2_lo"] = _pack_rc(lo, 11)
        # KV: [4096, 512] -> [4 supertiles, 128 part(=token%128), 8, 512]
        m["kc"] = np.ascontiguousarray(
            kc[:, qs].astype(np.float16).reshape(4, 8, 128, QKV_N)
            .transpose(0, 2, 1, 3))
        m["vc"] = np.ascontiguousarray(
            vc[:, qs].astype(np.float16).reshape(4, 8, 128, QKV_N)
            .transpose(0, 2, 1, 3))
        in_maps.append(m)
    return in_maps


def kernel(**inputs):
    nc = _build()
    in_maps = _shard(inputs)
    res = bass_utils.run_bass_kernel_spmd(
        nc, in_maps, core_ids=list(range(N_CORES)))
    return res.results[0]["y"]


# revision 51
# speedup vs baseline: 1.1492x; 1.0167x over previous
"""Llama decode block (single token) on 8 TRN2 NeuronCores, tensor-parallel.

Sharding (per sharding_hint): w_q/w_k/w_v/w_ff1 column-sharded, w_o/w_ff2
row-sharded, KV cache sharded by head (4 heads/core). AllReduce after the
attention output projection and after w_ff2. Residuals (x, x2) are added
locally after each AllReduce, so the AR payloads carry only matvec partials.

Memory-bound problem -> minimize HBM bytes and DMA descriptor count:

* Every weight is split on the host into a float16 "hi" part plus a
  float8e4 "lo" part holding 2^12*(W - hi).  3 bytes/elem instead of 4,
  with ~1e-5 effective relative error (fp16 residual quantized by fp8).
* All tensors are pre-packed on the host into the exact SBUF tile layout
  ([128 partitions, r, cols], row g(p,r)=r*128+p), so every DMA moves
  fully contiguous 16-32KB per-partition lines (the f32 baseline was
  descriptor-rate-bound at ~181 GB/s on 2KB strided descriptors).
* KV cache is fp16 (fp16 scores/attn keeps max rel err ~6e-3 vs the 2e-2
  gate; bf16 fails it).

Matvec scheme (per k-block): activation h is kept as an fp16 dual
(h1=fp16(h), h2=fp16(h-h1)) -> matmul with 2 stationary columns gives
psum rows [h1@Whi ; h2@Whi] at no extra PE cost; the lo pass uses
hs=fp16(h*2^-12) against Wlo=fp8(2^12*residual) and accumulates onto row
0, so products come out correctly scaled with no fixup pass. Row pairs
are combined either by the next matmul's contraction (q replicate, v
new-token), by PE transposes (ff1 -> silu columns), or by a SWDGE
accumulate DMA when writing the AllReduce input (wo, ff2).
"""

import math

import numpy as np
import ml_dtypes

import concourse.bass as bass
import concourse.mybir as mybir
import concourse.tile as tile
from concourse import bacc
from concourse import bass_utils

F32 = mybir.dt.float32
F16 = mybir.dt.float16
FP8 = mybir.dt.float8e4
AF = mybir.ActivationFunctionType
ALU = mybir.AluOpType

HIDDEN = 4096
N_HEADS = 32
HEAD_DIM = 128
INTERM = 11008
KV_LEN = 4096
N_CORES = 8

HEADS_PC = N_HEADS // N_CORES          # 4 heads per core
QKV_N = HEADS_PC * HEAD_DIM            # 512
FF_N = INTERM // N_CORES               # 1376
FF_NP = 1408                           # padded to 11*128
KB = HIDDEN // 128                     # 32 k-blocks of the hidden dim
SCALE = 1.0 / math.sqrt(HEAD_DIM)
LO = 2.0 ** 12                         # residual scale for the fp8 stream
ILO = 1.0 / LO

NP_FP8 = ml_dtypes.float8_e4m3

DEBUG = False          # emit intermediate tensors as extra outputs
WARM = False           # PE warm-up bursts
USE_TTR = False        # fused multiply-reduce for attention scores

# weight-stream tiling (r-blocks per DMA tile)
QKV_RT = [16, 16]                      # 32 kb in 2 tiles of [128,16,512]
WO_RT = [2, 2]                         # 4 r in 2 tiles of [128,2,4096]
FF1_RT = [6, 6, 6, 6, 6, 2]            # 32 kb over [128,r,1408] tiles
FF2_RT = [2, 2, 2, 2, 2, 1]            # 11 kb over [128,r,4096] tiles
FF1_CH = [(0, 512), (512, 512), (1024, 384)]   # ff1 psum column chunks


def _emit(nc, tc):
    i = {}

    def din(name, shape, dt=F32):
        i[name] = nc.dram_tensor(name, list(shape), dt, kind="ExternalInput").ap()

    din("x", [HIDDEN])
    din("attn_norm", [HIDDEN])
    din("ffn_norm", [HIDDEN])
    din("sinq", [2, 64])               # pre-scaled by 1/sqrt(d)
    din("cosq", [2, 64])
    din("sink", [2, 64])
    din("cosk", [2, 64])
    din("ident32", [32, 32])
    for w in ("wq", "wk", "wv"):
        din(w + "_hi", [128, KB, QKV_N], F16)
        din(w + "_lo", [128, KB, QKV_N], FP8)
    din("wo_hi", [128, HEADS_PC, HIDDEN], F16)
    din("wo_lo", [128, HEADS_PC, HIDDEN], FP8)
    din("wf1_hi", [128, KB, FF_NP], F16)
    din("wf1_lo", [128, KB, FF_NP], FP8)
    din("wf2_hi", [128, 11, HIDDEN], F16)
    din("wf2_lo", [128, 11, HIDDEN], FP8)
    din("kc", [4, 128, 8, QKV_N], F16)
    din("vc", [4, 128, 8, QKV_N], F16)
    y = nc.dram_tensor("y", [HIDDEN], F32, kind="ExternalOutput").ap()

    dbg_outs = {}

    def dbg(name, src_ap, shape):
        if not DEBUG:
            return
        d = nc.dram_tensor("dbg_" + name, list(shape), F32,
                           kind="ExternalOutput").ap()
        nc.sync.dma_start(d, src_ap)
        dbg_outs[name] = d

    with (
        tc.tile_pool(name="const", bufs=1) as cpool,
        tc.tile_pool(name="whi", bufs=5) as hpool,
        tc.tile_pool(name="wlo", bufs=4) as lpool,
        tc.tile_pool(name="kpool", bufs=2) as kpool,
        tc.tile_pool(name="vpool", bufs=2) as vpool,
        tc.tile_pool(name="sm", bufs=1) as sm,
        tc.tile_pool(name="scr", bufs=2) as scr,
        tc.tile_pool(name="psum", bufs=8, space="PSUM") as pp,
        tc.tile_pool(name="dram", bufs=1, space="DRAM") as dram,
    ):
        # ---- constants ----
        ones32 = cpool.tile([32, 1], F32)
        nc.vector.memset(ones32[:], 1.0)
        ones128 = cpool.tile([128, 1], F32)
        nc.vector.memset(ones128[:], 1.0)
        ones_r32 = cpool.tile([1, 32], F32)
        nc.vector.memset(ones_r32[:], 1.0)
        ones_r128 = cpool.tile([1, 128], F32)
        nc.vector.memset(ones_r128[:], 1.0)
        ones2_128 = cpool.tile([2, 128], F32)
        nc.vector.memset(ones2_128[:], 1.0)
        ones2_1 = cpool.tile([2, 1], F32)
        nc.vector.memset(ones2_1[:], 1.0)
        ones1_2 = cpool.tile([1, 2], F32)
        nc.vector.memset(ones1_2[:], 1.0)
        eps11 = cpool.tile([1, 1], F32)
        nc.vector.memset(eps11[:], 1e-6)
        ident32 = cpool.tile([32, 32], F32)
        nc.sync.dma_start(ident32[:], i["ident32"])
        trig = {}
        for t in ("sinq", "cosq", "sink", "cosk"):
            trig[t] = cpool.tile([2, 64], F32, name=t)
            nc.sync.dma_start(trig[t][:], i[t])

        # priming AllReduce: the first collective pays ~30us of ncfw warmup
        # + cross-core launch skew; burn that under the qkv weight stream so
        # the real AR1 runs at the ~12us floor.
        prime_in = dram.tile([8], F32, name="prime_in")
        prime_out = dram.tile([8], F32, name="prime_out")
        prime_sb = cpool.tile([1, 8], F32, name="prime_sb")
        nc.vector.memset(prime_sb[:], 0.0)
        nc.gpsimd.dma_start(prime_in[:].rearrange("(a d) -> a d", a=1),
                            prime_sb[:])
        nc.gpsimd.collective_compute(
            "AllReduce", ALU.add,
            replica_groups=[list(range(N_CORES))],
            ins=[prime_in[:].opt()], outs=[prime_out[:].opt()],
        )

        x_rows = cpool.tile([32, 128], F32)
        nc.sync.dma_start(x_rows[:], i["x"].rearrange("(a d) -> a d", a=32))
        anorm_rows = cpool.tile([32, 128], F32)
        nc.sync.dma_start(anorm_rows[:],
                          i["attn_norm"].rearrange("(a d) -> a d", a=32))
        fnorm_rows = cpool.tile([32, 128], F32)
        nc.sync.dma_start(fnorm_rows[:],
                          i["ffn_norm"].rearrange("(a d) -> a d", a=32))

        # ---- rmsnorm -> fp16 dual columns hd[128,32,2], hs[128,32] ----
        def rmsnorm_dual(xr, nr, tag):
            sq = sm.tile([32, 128], F32, name=f"sq_{tag}")
            ssq = sm.tile([32, 1], F32, name=f"ssq_{tag}")
            nc.scalar.activation(sq[:], xr[:], AF.Square, accum_out=ssq[:])
            ms_ps = pp.tile([1, 1], F32, name=f"ms_{tag}", tag="ps")
            nc.tensor.matmul(ms_ps[:], ones32[:], ssq[:])
            rstd = sm.tile([1, 1], F32, name=f"rstd_{tag}")
            nc.scalar.activation(rstd[:], ms_ps[:], AF.Sqrt,
                                 bias=eps11[:], scale=1.0 / HIDDEN)
            nc.vector.reciprocal(rstd[:], rstd[:])
            rstd_ps = pp.tile([32, 1], F32, name=f"rstdp_{tag}", tag="ps")
            nc.tensor.matmul(rstd_ps[:], ones_r32[:], rstd[:])
            rstd32 = sm.tile([32, 1], F32, name=f"rstd32_{tag}")
            nc.vector.tensor_copy(rstd32[:], rstd_ps[:])
            h_rows = sm.tile([32, 128], F32, name=f"hr_{tag}")
            nc.vector.tensor_tensor(h_rows[:], xr[:], nr[:], ALU.mult)
            nc.vector.tensor_scalar_mul(h_rows[:], h_rows[:], rstd32[:])
            h_ps = pp.tile([128, 32], F32, name=f"hps_{tag}", tag="ps")
            nc.tensor.transpose(h_ps[:], h_rows[:], ident32[:])
            h_cols = sm.tile([128, 32], F32, name=f"hc_{tag}")
            nc.vector.tensor_copy(h_cols[:], h_ps[:])
            hd = sm.tile([128, 32, 2], F16, name=f"hd_{tag}")
            nc.vector.tensor_copy(hd[:, :, 0], h_cols[:])
            tmp = sm.tile([128, 32], F32, name=f"htmp_{tag}")
            nc.vector.tensor_tensor(tmp[:], h_cols[:], hd[:, :, 0], ALU.subtract)
            nc.vector.tensor_copy(hd[:, :, 1], tmp[:])
            hs = sm.tile([128, 32], F16, name=f"hs_{tag}")
            nc.vector.tensor_scalar_mul(hs[:], h_cols[:], ILO)
            return hd, hs

        hd, hs = rmsnorm_dual(x_rows, anorm_rows, "a")

        # PE warm-up: the HAM clock gate releases (1.2 -> 2.4 GHz) only
        # after ~3.4us of sustained PE activity.  Burn dummy matmuls into a
        # scratch bank at points where the PE would otherwise sit idle
        # (kernel entry barrier, attention DVE phase, AllReduce waits) so
        # the real matvec streams run at full clock.
        warm_in = cpool.tile([128, 512], F16, name="warm_in")
        nc.vector.memset(warm_in[:], 0.0)

        def pe_warm(tag, count, stat):
            wps = pp.tile([2, 512], F32, name=f"warm_{tag}", tag="ps")
            for it in range(count):
                nc.tensor.matmul(wps[:], stat, warm_in[:],
                                 start=(it == 0), stop=(it == count - 1))
            sink = sm.tile([2, 1], F32, name=f"wsink_{tag}")
            nc.vector.tensor_copy(sink[:], wps[:, 0:1])

        if WARM:
            pe_warm("a", 20, hd[:, 0, :])

        # ---- q/k/v: one psum bank per projection (one accumulation group
        # per bank); lo accumulates on row 0, dual correction on row 1 ----
        qkv_ps = {w: pp.tile([2, QKV_N], F32, name=f"{w}_ps", tag="ps")
                  for w in ("wq", "wk", "wv")}
        for wi, w in enumerate(("wq", "wk", "wv")):
            ps = qkv_ps[w]
            kb0 = 0
            for t, rt in enumerate(QKV_RT):
                hi_t = hpool.tile([128, rt, QKV_N], F16, name="qkv_hi", tag="whi")
                nc.sync.dma_start(hi_t[:], i[w + "_hi"][:, kb0:kb0 + rt, :])
                lo_t = lpool.tile([128, rt, QKV_N], FP8, name="qkv_lo", tag="wlo")
                nc.sync.dma_start(lo_t[:], i[w + "_lo"][:, kb0:kb0 + rt, :])
                for b in range(rt):
                    kb = kb0 + b
                    nc.tensor.matmul(
                        ps[0:2, :], hd[:, kb, :], hi_t[:, b, :],
                        start=(kb == 0), stop=False,
                    )
                    nc.tensor.matmul(
                        ps[0:1, :], hs[:, kb:kb + 1], lo_t[:, b, :],
                        start=False, stop=(kb == KB - 1),
                    )
                kb0 += rt

        q_sb = sm.tile([2, QKV_N], F32, name="q_sb")
        nc.vector.tensor_copy(q_sb[:], qkv_ps["wq"][:])
        k_sb = sm.tile([2, QKV_N], F32, name="k_sb")
        nc.vector.tensor_copy(k_sb[:], qkv_ps["wk"][:])
        v16 = sm.tile([2, QKV_N], F16, name="v16")
        nc.vector.tensor_copy(v16[:], qkv_ps["wv"][:])

        # ---- RoPE (dual rows; q uses trig pre-scaled by 1/sqrt(d)) ----
        def rope(src, sin_t, cos_t, tag):
            out = sm.tile([2, QKV_N], F32, name=f"rope_{tag}")
            tmp = sm.tile([2, QKV_N], F32, name=f"ropetmp_{tag}")
            r3 = src[:].rearrange("p (h d) -> p h d", h=HEADS_PC)
            o3 = out[:].rearrange("p (h d) -> p h d", h=HEADS_PC)
            t3 = tmp[:].rearrange("p (h d) -> p h d", h=HEADS_PC)
            cb = cos_t[:].unsqueeze(1).to_broadcast((2, HEADS_PC, 64))
            sb = sin_t[:].unsqueeze(1).to_broadcast((2, HEADS_PC, 64))
            x1, x2 = r3[:, :, 0:64], r3[:, :, 64:128]
            nc.vector.tensor_tensor(o3[:, :, 0:64], x1, cb, ALU.mult)
            nc.vector.tensor_tensor(t3[:, :, 0:64], x2, sb, ALU.mult)
            nc.vector.tensor_sub(o3[:, :, 0:64], o3[:, :, 0:64],
                                 t3[:, :, 0:64])
            nc.vector.tensor_tensor(o3[:, :, 64:128], x2, cb, ALU.mult)
            nc.vector.tensor_tensor(t3[:, :, 64:128], x1, sb, ALU.mult)
            nc.vector.tensor_add(o3[:, :, 64:128], o3[:, :, 64:128],
                                 t3[:, :, 64:128])
            return out

        dbg("q_sb", q_sb[:], [2, QKV_N])
        dbg("k_sb", k_sb[:], [2, QKV_N])

        rope_q = rope(q_sb, trig["sinq"], trig["cosq"], "q")
        rope_k = rope(k_sb, trig["sink"], trig["cosk"], "k")
        dbg("rope_q", rope_q[:], [2, QKV_N])

        # q replicated to 128 partitions; the ones-matmul also sums the dual
        qrep_ps = pp.tile([128, QKV_N], F32, name="qrep_ps", tag="ps")
        nc.tensor.matmul(qrep_ps[:], ones2_128[:], rope_q[:])
        q_rep = sm.tile([128, QKV_N], F16, name="q_rep")
        nc.vector.tensor_copy(q_rep[:], qrep_ps[:])
        q_rep32 = sm.tile([2, QKV_N], F32, name="q_rep32")
        nc.vector.tensor_copy(q_rep32[:], qrep_ps[0:2, :])

        # current-token score: combine k dual via ones-matmul, then q.k
        kc_ps = pp.tile([1, QKV_N], F32, name="kc_ps", tag="ps")
        nc.tensor.matmul(kc_ps[:], ones2_1[:], rope_k[:])
        k_comb = sm.tile([1, QKV_N], F32, name="k_comb")
        nc.vector.tensor_copy(k_comb[:], kc_ps[:])
        prod_new = sm.tile([1, QKV_N], F32, name="prod_new")
        nc.vector.tensor_tensor(prod_new[:], k_comb[:], q_rep32[0:1, :],
                                ALU.mult)
        s_new = sm.tile([1, HEADS_PC], F32, name="s_new")
        nc.vector.tensor_reduce(
            s_new[:], prod_new[:].rearrange("p (h d) -> p h d", h=HEADS_PC),
            mybir.AxisListType.X, ALU.add)
        e_new = sm.tile([1, HEADS_PC], F32, name="e_new")
        nc.scalar.activation(e_new[:], s_new[:], AF.Exp)
        e2_ps = pp.tile([2, HEADS_PC], F32, name="e2_ps", tag="ps")
        nc.tensor.matmul(e2_ps[:], ones1_2[:], e_new[:])
        e_new2 = sm.tile([2, HEADS_PC], F16, name="e_new2")
        nc.vector.tensor_copy(e_new2[:], e2_ps[:])

        # ---- attention over the KV cache ----
        o_ps = pp.tile([128, HEADS_PC], F32, name="o_ps", tag="ps")
        denom_acc = sm.tile([128, HEADS_PC], F32, name="denom_acc")
        nc.vector.memset(denom_acc[:], 0.0)

        for st in range(4):
            k_sup = kpool.tile([128, 8, QKV_N], F16, name="k_sup", tag="k")
            v_sup = vpool.tile([128, 8, QKV_N], F16, name="v_sup", tag="v")
            nc.sync.dma_start(k_sup[:], i["kc"][st])
            nc.sync.dma_start(v_sup[:], i["vc"][st])
            for b in range(8):
                scores = scr.tile([128, HEADS_PC], F32, name="scores", tag="sc")
                if USE_TTR:
                    prod = scr.tile([128, 128], F16, name="prod", tag="prod")
                    for h in range(HEADS_PC):
                        nc.vector.tensor_tensor_reduce(
                            prod[:], k_sup[:, b, h * 128:(h + 1) * 128],
                            q_rep[:, h * 128:(h + 1) * 128],
                            1.0, 0.0, ALU.mult, ALU.add, scores[:, h:h + 1])
                else:
                    prod = scr.tile([128, QKV_N], F16, name="prod", tag="prod")
                    nc.vector.tensor_tensor(prod[:], k_sup[:, b, :], q_rep[:],
                                            ALU.mult)
                    nc.vector.tensor_reduce(
                        scores[:],
                        prod[:].rearrange("p (h d) -> p h d", h=HEADS_PC),
                        mybir.AxisListType.X, ALU.add)
                expt = scr.tile([128, HEADS_PC], F32, name="expt", tag="ex")
                nc.scalar.activation(expt[:], scores[:], AF.Exp)
                nc.vector.tensor_add(denom_acc[:], denom_acc[:], expt[:])
                expt16 = scr.tile([128, HEADS_PC], F16, name="expt16", tag="e16")
                nc.scalar.copy(expt16[:], expt[:])
                for h in range(HEADS_PC):
                    nc.tensor.matmul(
                        o_ps[:, h:h + 1],
                        v_sup[:, b, h * 128:(h + 1) * 128],
                        expt16[:, h:h + 1],
                        start=(st == 0 and b == 0 and h == 0), stop=False,
                    )
        for h in range(HEADS_PC):
            nc.tensor.matmul(
                o_ps[:, h:h + 1], v16[:, h * 128:(h + 1) * 128],
                e_new2[:, h:h + 1],
                start=False, stop=(h == HEADS_PC - 1),
            )

        # denom = sum over tokens of the same fp16 expt + e_new
        d_ps = pp.tile([1, HEADS_PC], F32, name="d_ps", tag="ps")
        nc.tensor.matmul(d_ps[:], ones128[:], denom_acc[:])
        denom = sm.tile([1, HEADS_PC], F32, name="denom")
        nc.vector.tensor_copy(denom[:], d_ps[:])
        nc.vector.tensor_add(denom[:], denom[:], e_new[:])
        nc.vector.reciprocal(denom[:], denom[:])
        r_ps = pp.tile([128, HEADS_PC], F32, name="r_ps", tag="ps")
        nc.tensor.matmul(r_ps[:], ones_r128[:], denom[:])
        recip_bc = sm.tile([128, HEADS_PC], F32, name="recip_bc")
        nc.vector.tensor_copy(recip_bc[:], r_ps[:])
        o_sb = sm.tile([128, HEADS_PC], F32, name="o_sb")
        nc.vector.tensor_tensor(o_sb[:], o_ps[:], recip_bc[:], ALU.mult)
        dbg("denom", denom[:], [1, HEADS_PC])
        dbg("o_sb", o_sb[:], [128, HEADS_PC])

        od = sm.tile([128, HEADS_PC, 2], F16, name="od")
        nc.vector.tensor_copy(od[:, :, 0], o_sb[:])
        otmp = sm.tile([128, HEADS_PC], F32, name="otmp")
        nc.vector.tensor_tensor(otmp[:], o_sb[:], od[:, :, 0], ALU.subtract)
        nc.vector.tensor_copy(od[:, :, 1], otmp[:])
        os_ = sm.tile([128, HEADS_PC], F16, name="os_")
        nc.vector.tensor_scalar_mul(os_[:], o_sb[:], ILO)

        # ---- o @ w_o: 8 output chunks, one psum bank each ----
        if WARM:
            pe_warm("b", 10, od[:, 0, :])
        wo_ps = [pp.tile([2, 512], F32, name=f"wo_ps{n}", tag="ps")
                 for n in range(8)]
        r0 = 0
        for t, rt in enumerate(WO_RT):
            hi_t = hpool.tile([128, rt, HIDDEN], F16, name="wo_hi", tag="whi")
            nc.sync.dma_start(hi_t[:], i["wo_hi"][:, r0:r0 + rt, :])
            lo_t = lpool.tile([128, rt, HIDDEN], FP8, name="wo_lo", tag="wlo")
            nc.sync.dma_start(lo_t[:], i["wo_lo"][:, r0:r0 + rt, :])
            for b in range(rt):
                r = r0 + b
                for n in range(8):
                    nc.tensor.matmul(
                        wo_ps[n][0:2, :], od[:, r, :],
                        hi_t[:, b, 512 * n:512 * n + 512],
                        start=(r == 0), stop=False,
                    )
                    nc.tensor.matmul(
                        wo_ps[n][0:1, :], os_[:, r:r + 1],
                        lo_t[:, b, 512 * n:512 * n + 512],
                        start=False, stop=(r == HEADS_PC - 1),
                    )
            r0 += rt
        # stage as [2, 4096]: row 0 = hi+lo part, row 1 = dual correction;
        # the SWDGE pair below writes row 0 then accumulates row 1 on DRAM
        wo_sb = sm.tile([2, HIDDEN], F32, name="ar_stage", tag="ar_stage")
        for n in range(8):
            eng = nc.vector.tensor_copy if n % 2 == 0 else nc.scalar.copy
            eng(wo_sb[0:2, 512 * n:512 * n + 512], wo_ps[n][:])

        dbg("wo_sb", wo_sb[:], [2, HIDDEN])
        ar1_in = dram.tile([HIDDEN], F32, name="ar1_in")
        ar1_out = dram.tile([HIDDEN], F32, name="ar1_out")
        ar1v = ar1_in[:].rearrange("(a d) -> a d", a=1)
        nc.gpsimd.dma_start(ar1v, wo_sb[0:1, :])
        nc.gpsimd.dma_start(ar1v, wo_sb[1:2, :], accum_op=ALU.add)
        nc.gpsimd.collective_compute(
            "AllReduce", ALU.add,
            replica_groups=[list(range(N_CORES))],
            ins=[ar1_in[:].opt()], outs=[ar1_out[:].opt()],
        )

        # ---- MLP ----
        ar1_rows = sm.tile([32, 128], F32, name="ar1_rows")
        nc.sync.dma_start(ar1_rows[:], ar1_out[:].rearrange("(a d) -> a d", a=32))
        x2_rows = sm.tile([32, 128], F32, name="x2_rows")
        nc.vector.tensor_add(x2_rows[:], x_rows[:], ar1_rows[:])
        dbg("x2_rows", x2_rows[:], [32, 128])

        hd2, hs2 = rmsnorm_dual(x2_rows, fnorm_rows, "b")
        if WARM:
            pe_warm("c", 10, hd2[:, 0, :])

        f1_ps = [pp.tile([2, 512], F32, name=f"f1_ps{n}", tag="ps")
                 for n in range(3)]
        kb0 = 0
        for t, rt in enumerate(FF1_RT):
            hi_t = hpool.tile([128, 6, FF_NP], F16, name="f1_hi", tag="whi")
            lo_t = lpool.tile([128, 6, FF_NP], FP8, name="f1_lo", tag="wlo")
            nc.sync.dma_start(hi_t[:, 0:rt, :], i["wf1_hi"][:, kb0:kb0 + rt, :])
            nc.sync.dma_start(lo_t[:, 0:rt, :], i["wf1_lo"][:, kb0:kb0 + rt, :])
            for b in range(rt):
                kb = kb0 + b
                for n, (c0, w) in enumerate(FF1_CH):
                    nc.tensor.matmul(
                        f1_ps[n][0:2, 0:w], hd2[:, kb, :],
                        hi_t[:, b, c0:c0 + w],
                        start=(kb == 0), stop=False,
                    )
                    nc.tensor.matmul(
                        f1_ps[n][0:1, 0:w], hs2[:, kb:kb + 1],
                        lo_t[:, b, c0:c0 + w],
                        start=False, stop=(kb == KB - 1),
                    )
            kb0 += rt
        f1_sb = [sm.tile([2, 512], F32, name=f"f1_sb{n}") for n in range(3)]
        for n in range(3):
            nc.vector.tensor_copy(f1_sb[n][:], f1_ps[n][:])

        # a-columns via PE transposes of each dual pair (128-col windows)
        acol_ps = pp.tile([128, 22], F32, name="acol_ps", tag="ps")
        ident2 = ident32[0:2, 0:2]
        for j in range(11):
            n = (128 * j) // 512
            off = 128 * j - 512 * n
            nc.tensor.transpose(acol_ps[:, 2 * j:2 * j + 2],
                                f1_sb[n][:, off:off + 128], ident2)
        acol_sb = sm.tile([128, 22], F32, name="acol_sb")
        nc.vector.tensor_copy(acol_sb[:], acol_ps[:])
        pre = sm.tile([128, 11], F32, name="pre_silu")
        a3 = acol_sb[:].rearrange("p (j t) -> p j t", t=2)
        nc.vector.tensor_tensor(pre[:], a3[:, :, 0], a3[:, :, 1], ALU.add)
        sig = sm.tile([128, 11], F32, name="sig")
        nc.scalar.activation(sig[:], pre[:], AF.Sigmoid)
        a_sb = sm.tile([128, 11], F32, name="a_sb")
        nc.vector.tensor_tensor(a_sb[:], pre[:], sig[:], ALU.mult)
        dbg("a_sb", a_sb[:], [128, 11])
        ad = sm.tile([128, 11, 2], F16, name="ad")
        nc.vector.tensor_copy(ad[:, :, 0], a_sb[:])
        atmp = sm.tile([128, 11], F32, name="atmp")
        nc.vector.tensor_tensor(atmp[:], a_sb[:], ad[:, :, 0], ALU.subtract)
        nc.vector.tensor_copy(ad[:, :, 1], atmp[:])
        as_ = sm.tile([128, 11], F16, name="as_")
        nc.vector.tensor_scalar_mul(as_[:], a_sb[:], ILO)

        f2_ps = [pp.tile([2, 512], F32, name=f"f2_ps{n}", tag="ps")
                 for n in range(8)]
        kb0 = 0
        for t, rt in enumerate(FF2_RT):
            hi_t = hpool.tile([128, rt, HIDDEN], F16, name="f2_hi", tag="whi")
            nc.sync.dma_start(hi_t[:], i["wf2_hi"][:, kb0:kb0 + rt, :])
            lo_t = lpool.tile([128, rt, HIDDEN], FP8, name="f2_lo", tag="wlo")
            nc.sync.dma_start(lo_t[:], i["wf2_lo"][:, kb0:kb0 + rt, :])
            for b in range(rt):
                kb = kb0 + b
                for n in range(8):
                    nc.tensor.matmul(
                        f2_ps[n][0:2, :], ad[:, kb, :],
                        hi_t[:, b, 512 * n:512 * n + 512],
                        start=(kb == 0), stop=False,
                    )
                    nc.tensor.matmul(
                        f2_ps[n][0:1, :], as_[:, kb:kb + 1],
                        lo_t[:, b, 512 * n:512 * n + 512],
                        start=False, stop=(kb == 10),
                    )
            kb0 += rt
        ff_sb = sm.tile([2, HIDDEN], F32, name="ff_stage", tag="ar_stage")
        for n in range(8):
            eng = nc.vector.tensor_copy if n % 2 == 0 else nc.scalar.copy
            eng(ff_sb[0:2, 512 * n:512 * n + 512], f2_ps[n][:])

        ar2_in = dram.tile([HIDDEN], F32, name="ar2_in")
        ar2_out = dram.tile([HIDDEN], F32, name="ar2_out")
        ar2v = ar2_in[:].rearrange("(a d) -> a d", a=1)
        nc.gpsimd.dma_start(ar2v, ff_sb[0:1, :])
        nc.gpsimd.dma_start(ar2v, ff_sb[1:2, :], accum_op=ALU.add)
        nc.gpsimd.collective_compute(
            "AllReduce", ALU.add,
            replica_groups=[list(range(N_CORES))],
            ins=[ar2_in[:].opt()], outs=[ar2_out[:].opt()],
        )

        ar2_rows = sm.tile([32, 128], F32, name="ar2_rows")
        nc.sync.dma_start(ar2_rows[:], ar2_out[:].rearrange("(a d) -> a d", a=32))
        y_rows = sm.tile([32, 128], F32, name="y_rows")
        nc.vector.tensor_add(y_rows[:], x2_rows[:], ar2_rows[:])
        nc.sync.dma_start(y.rearrange("(a d) -> a d", a=32), y_rows[:])


_BUILT = None


def _build():
    global _BUILT
    if _BUILT is None:
        nc = bacc.Bacc("TRN2", target_bir_lowering=False, debug=False,
                       num_devices=N_CORES)
        with tile.TileContext(nc) as tc:
            _emit(nc, tc)
        nc.compile()
        _BUILT = nc
    return _BUILT


def _hilo(W):
    hi = W.astype(np.float16)
    res = (W - hi.astype(np.float32)) * LO
    lo = np.clip(res, -224.0, 224.0).astype(NP_FP8)
    return hi, lo


def _pack_rc(A, r128):
    """[r128*128, C] -> [128, r128, C] with row r*128+p on partition p."""
    C = A.shape[1]
    return np.ascontiguousarray(A.reshape(r128, 128, C).transpose(1, 0, 2))


def _shard(inputs):
    f = lambda a: np.ascontiguousarray(np.asarray(a, dtype=np.float32))
    x = f(inputs["x"])
    attn_norm = f(inputs["attn_norm"])
    ffn_norm = f(inputs["ffn_norm"])
    pos = int(np.asarray(inputs["pos"]))
    sin = f(inputs["sin_cache"][pos])
    cos = f(inputs["cos_cache"][pos])
    sinq = np.ascontiguousarray(np.stack([sin * SCALE] * 2).astype(np.float32))
    cosq = np.ascontiguousarray(np.stack([cos * SCALE] * 2).astype(np.float32))
    sink = np.ascontiguousarray(np.stack([sin] * 2).astype(np.float32))
    cosk = np.ascontiguousarray(np.stack([cos] * 2).astype(np.float32))
    wq, wk, wv = f(inputs["w_q"]), f(inputs["w_k"]), f(inputs["w_v"])
    wo, wf1, wf2 = f(inputs["w_o"]), f(inputs["w_ff1"]), f(inputs["w_ff2"])
    kc = f(inputs["k_cache"]).reshape(KV_LEN, N_HEADS * HEAD_DIM)
    vc = f(inputs["v_cache"]).reshape(KV_LEN, N_HEADS * HEAD_DIM)
    ident32 = np.eye(32, dtype=np.float32)

    in_maps = []
    for c in range(N_CORES):
        qs = slice(c * QKV_N, (c + 1) * QKV_N)
        fs = slice(c * FF_N, (c + 1) * FF_N)
        m = {
            "x": x, "attn_norm": attn_norm, "ffn_norm": ffn_norm,
            "sinq": sinq, "cosq": cosq, "sink": sink, "cosk": cosk,
            "ident32": ident32,
        }
        for name, wfull in (("wq", wq), ("wk", wk), ("wv", wv)):
            hi, lo = _hilo(wfull[:, qs])
            m[name + "_hi"] = _pack_rc(hi, KB)
            m[name + "_lo"] = _pack_rc(lo, KB)
        hi, lo = _hilo(wo[qs, :])
        m["wo_hi"] = _pack_rc(hi, HEADS_PC)
        m["wo_lo"] = _pack_rc(lo, HEADS_PC)
        w1 = np.pad(wf1[:, fs], ((0, 0), (0, FF_NP - FF_N)))
        hi, lo = _hilo(w1)
        m["wf1_hi"] = _pack_rc(hi, KB)
        m["wf1_lo"] = _pack_rc(lo, KB)
        w2 = np.pad(wf2[fs, :], ((0, FF_NP - FF_N), (0, 0)))
        hi, lo = _hilo(w2)
        m["wf2_hi"] = _pack_rc(hi, 11)
        m["wf2_lo"] = _pack_rc(lo, 11)
        # KV: [4096, 512] -> [4 supertiles, 128 part(=token%128), 8, 512]
        m["kc"] = np.ascontiguousarray(
            kc[:, qs].astype(np.float16).reshape(4, 8, 128, QKV_N)
            .transpose(0, 2, 1, 3))
        m["vc"] = np.ascontiguousarray(
            vc[:, qs].astype(np.float16).reshape(4, 8, 128, QKV_N)
            .transpose(0, 2, 1, 3))
        in_maps.append(m)
    return in_maps


def kernel(**inputs):
    nc = _build()
    in_maps = _shard(inputs)
    res = bass_utils.run_bass_kernel_spmd(
        nc, in_maps, core_ids=list(range(N_CORES)))
    return res.results[0]["y"]
